# revision 20
# baseline (speedup 1.0000x reference)
"""Trainium2 Bass kernel for nn_Attention_62620623176132.

Multi-head causal attention with RoPE (LLaMA-style), B=2, S=2048, D=2048,
H=16 heads of HD=128, fp32 reference.

Sharding (hardcoded): 8 cores = 2-way data parallel over batch x 4-way
tensor parallel over heads (4 heads per core). Each core computes its 4
heads' Q/K/V projections, attention, and a partial output projection
(rows of wo for its heads); the host sums the 4 fp16 partials per batch
in fp32.

Device algorithm (per core; matmuls in fp16 with fp32 PSUM accumulation):
  - x^T kept SBUF-resident; Q^T/K^T computed per head in [HD, S] layout,
    V in [S, dv] layout, so no transposes are ever needed.
  - RoPE via host-side even/odd column permutation of wq/wk: rotation
    pairs land in partition halves; 3 DVE tensor ops + 2 swap copies.
  - Scores computed transposed, sT[kt, qt] = kT . qT, so exp(sT) feeds
    the PV matmul directly as the moving operand. exp is shifted by -4
    (softmax is shift-invariant) to keep fp16 outputs far from overflow.
  - Softmax denominators: exp tiles are summed on the Vector engine and
    one all-ones stationary matmul per query chunk broadcasts the column
    sums to all partitions; the normalization multiply is fused into the
    PSUM->SBUF copy of the attention output.
  - Causality: score tiles above the diagonal are skipped; band tiles
    are restricted to their unmasked columns and the diagonal square
    gets -60000 added in PSUM by one extra matmul (identity x triangle),
    so masking costs no vector-engine work.
  - Projections for head h+1 are emitted inside head h's attention so
    the serial RoPE chain never stalls the PE; the output projection is
    streamed inside the last head's attention the same way.
  - Startup DMAs are ordered/split so the first projection matmuls start
    as soon as the first x^T chunk lands.

Fallback paths keyed off the runtime mask: all-zero mask -> non-causal
kernel; any other mask -> multiplicative exp(mask/sqrt(HD)) tiles
streamed from DRAM (correct for arbitrary masks, slower).
"""

import math

import numpy as np
import ml_dtypes
import concourse.tile as tile
import concourse.mybir as mybir
from concourse import bacc
from concourse.bass import ts
from concourse.bass_utils import run_bass_kernel_spmd

B, S, D, H, HD = 2, 2048, 2048, 16, 128
P = 128
NCORES = 8
TP = 4                # head-parallel groups
HPC = H // TP         # heads per core = 4
DVC = HPC * HD        # 512 v-dims per core
KC = D // P           # 16 contraction chunks
NT = S // P           # 16 token tiles of 128
NQ = S // 512         # 4 query chunks of 512
F8 = mybir.dt.float8e4
F16 = mybir.dt.float16
F32 = mybir.dt.float32
NPF8 = ml_dtypes.float8_e4m3
NPF16 = np.float16
MASK_NEG = -60000.0
SCALE = 1.0 / math.sqrt(HD)
WS = 64.0             # fp8 weight pre-scale (power of 2, exact)
OS = 32.0             # OT8 scale, via oneq = 1/32 denominator operand
EXP = mybir.ActivationFunctionType.Exp
CPY = mybir.ActivationFunctionType.Copy
DR = mybir.MatmulPerfMode.DoubleRow

_cache: dict = {}


def _build_causal8():
    """fp8 causal kernel: query-chunk 0 (tokens 0-511) fully fp16 (outputs
    there reach ~40 sigma and need low relative error); queries 512+ use
    fp8e4 DoubleRow matmuls wherever the contraction doubles (projections,
    PV pairs, denominator pairs, output projection). Scores stay fp16:
    their contraction is HD=128 and DoubleRow cannot help (measured 1.0x).

    fp8 weights are pre-scaled x64 (exact); 1/64 descale is folded into
    PSUM->SBUF copies. OT8 carries x32 via the 1/32 'oneq' denominator
    matmul; the x2048 on fp8 output-projection PSUM descales in the final
    copy."""
    nc = bacc.Bacc("TRN2", target_bir_lowering=False, debug=False,
                   num_devices=NCORES)

    def din(name, shape, dt):
        return nc.dram_tensor(name, shape, dt, kind="ExternalInput").ap()

    xT16_d = din("xT16", [P, KC, 512], F16)
    xT8_d = din("xT8", [P, 3, KC, 512], F8)
    wq16_d = din("wq16", [P, HPC, KC, HD], F16)
    wk16_d = din("wk16", [P, HPC, KC, HD], F16)
    wq8_d = din("wq8", [P, HPC, KC, HD], F8)
    wk8_d = din("wk8", [P, HPC, KC, HD], F8)
    wv16_d = din("wv16", [P, KC, DVC], F16)
    wv8_d = din("wv8", [P, KC, DVC], F8)
    wo16_d = din("wo16", [P, HPC, D], F16)
    wo8_d = din("wo8", [P, HPC, D], F8)
    c2_d = din("c2", [P, S], F16)
    s2n_d = din("s2n", [P, S], F16)
    eye_d = din("eye", [P, P], F16)
    mtri_d = din("mtri", [P, P], F16)
    mtri2_d = din("mtri2", [P, 256], F16)
    ones_d = din("ones", [P, P], F16)
    oneq_d = din("oneq", [P, 2, P], F8)
    out_d = nc.dram_tensor("out", [P, NT, D], F16, kind="ExternalOutput").ap()

    with tile.TileContext(nc) as tc:
        with tc.tile_pool(name="static", bufs=1) as st, \
             tc.tile_pool(name="w1", bufs=1) as w1, \
             tc.tile_pool(name="w2", bufs=2) as w2, \
             tc.tile_pool(name="et", bufs=4) as etp, \
             tc.tile_pool(name="fo", bufs=4) as fop, \
             tc.tile_pool(name="rv", bufs=2) as rvp, \
             tc.tile_pool(name="pj", bufs=2, space="PSUM") as pjp:

            # ---- static tensors ------------------------------------------
            xT16 = st.tile([P, KC, 512], F16, tag="xT16")
            xT8 = st.tile([P, 3, KC, 512], F8, tag="xT8")
            wv16 = st.tile([P, KC, DVC], F16, tag="wv16")
            wv8 = st.tile([P, KC, DVC], F8, tag="wv8")
            wo16 = st.tile([P, HPC, D], F16, tag="wo16")
            wo8 = st.tile([P, HPC, D], F8, tag="wo8")
            c2 = st.tile([P, S], F16, tag="c2")
            s2n = st.tile([P, S], F16, tag="s2n")
            eye16 = st.tile([P, P], F16, tag="eye")
            mtri16 = st.tile([P, P], F16, tag="mtri")
            mtri2 = st.tile([P, 256], F16, tag="mtri2")
            ones16 = st.tile([P, P], F16, tag="ones")
            oneq = st.tile([P, 2, P], F8, tag="oneq")
            V16 = st.tile([P, 4, DVC], F16, tag="V16")
            V8 = st.tile([P, NT // 2, 2, DVC], F8, tag="V8")
            OT16 = st.tile([P, HPC, 512], F16, tag="OT16")
            OT8 = st.tile([P, HPC, S], F8, tag="OT8")
            bias4 = st.tile([P, 1], F32, tag="b4")
            bias2 = st.tile([P, 1], F32, tag="b2")
            nc.vector.memset(bias4[:], -4.0)
            nc.vector.memset(bias2[:], -2.0)

            # startup DMAs: corner-proj operands first, then the rest
            wq16_h = w1.tile([P, KC, HD], F16, tag="wq16h")
            wk16_h = w1.tile([P, KC, HD], F16, tag="wk16h")
            wq8_h = w1.tile([P, KC, HD], F8, tag="wq8h")
            wk8_h = w1.tile([P, KC, HD], F8, tag="wk8h")
            for g in range(4):
                nc.sync.dma_start(wq16_h[:, ts(g, 4), :],
                                  wq16_d[:, 0, ts(g, 4), :])
                nc.sync.dma_start(xT16[:, ts(g, 4), :], xT16_d[:, ts(g, 4), :])
            nc.sync.dma_start(wk16_h[:], wk16_d[:, 0])
            nc.sync.dma_start(wv16[:], wv16_d)
            nc.sync.dma_start(xT8[:], xT8_d)
            nc.sync.dma_start(wq8_h[:], wq8_d[:, 0])
            nc.sync.dma_start(wk8_h[:], wk8_d[:, 0])
            nc.sync.dma_start(wv8[:], wv8_d)
            nc.sync.dma_start(c2[:], c2_d)
            nc.sync.dma_start(s2n[:], s2n_d)
            nc.sync.dma_start(eye16[:], eye_d)
            nc.sync.dma_start(mtri16[:], mtri_d)
            nc.sync.dma_start(mtri2[:], mtri2_d)
            nc.sync.dma_start(ones16[:], ones_d)
            nc.sync.dma_start(oneq[:], oneq_d)
            nc.sync.dma_start(wo16[:], wo16_d)
            nc.sync.dma_start(wo8[:], wo8_d)

            def rope(raw, rot_tag):
                swp = w1.tile([P, S], F16, tag="swap")
                nc.vector.tensor_copy(swp[0:64, :], raw[64:128, :])
                nc.vector.tensor_copy(swp[64:128, :], raw[0:64, :])
                rot = w2.tile([P, S], F16, tag=rot_tag)
                nc.vector.tensor_mul(rot[:], raw[:], c2[:])
                nc.vector.tensor_mul(swp[:], swp[:], s2n[:])
                nc.vector.tensor_add(rot[:], rot[:], swp[:])
                return rot

            def proj_corner(w16h, raw):
                ps = pjp.tile([P, 512], F32, tag="pj")
                for kc in range(KC):
                    nc.tensor.matmul(ps[:], w16h[:, kc, :], xT16[:, kc, :],
                                     start=(kc == 0), stop=(kc == KC - 1))
                nc.scalar.copy(raw[:, 0:512], ps[:])

            def proj_fp8(w8h, raw, c):
                ps = pjp.tile([P, 512], F32, tag="pj")
                for k in range(KC // 2):
                    nc.tensor.matmul(ps[:], w8h[:, 2 * k:2 * k + 2, :],
                                     xT8[:, c - 1, 2 * k:2 * k + 2, :],
                                     start=(k == 0), stop=(k == KC // 2 - 1),
                                     perf_mode=DR)
                nc.vector.tensor_scalar_mul(raw[:, ts(c, 512)], ps[:], 1.0 / WS)

            def v_corner(ti):
                ps = pjp.tile([P, 512], F32, tag="pj")
                for kc in range(KC):
                    nc.tensor.matmul(ps[:], xT16[:, kc, ts(ti, P)],
                                     wv16[:, kc, :],
                                     start=(kc == 0), stop=(kc == KC - 1))
                nc.scalar.copy(V16[:, ti, :], ps[:])
                nc.scalar.copy(V8[:, ti // 2, ti % 2, :], ps[:])

            def v_fp8(ti):
                ps = pjp.tile([P, 512], F32, tag="pj")
                c = ti // 4
                for k in range(KC // 2):
                    nc.tensor.matmul(ps[:], xT8[:, c - 1, 2 * k:2 * k + 2,
                                                 ts(ti % 4, P)],
                                     wv8[:, 2 * k:2 * k + 2, :],
                                     start=(k == 0), stop=(k == KC // 2 - 1),
                                     perf_mode=DR)
                nc.scalar.activation(V8[:, ti // 2, ti % 2, :], ps[:], CPY,
                                     scale=1.0 / WS)

            def load_w(h):
                wq16h = w1.tile([P, KC, HD], F16, tag="wq16h")
                wk16h = w1.tile([P, KC, HD], F16, tag="wk16h")
                wq8h = w1.tile([P, KC, HD], F8, tag="wq8h")
                wk8h = w1.tile([P, KC, HD], F8, tag="wk8h")
                nc.sync.dma_start(wq16h[:], wq16_d[:, h])
                nc.sync.dma_start(wk16h[:], wk16_d[:, h])
                nc.sync.dma_start(wq8h[:], wq8_d[:, h])
                nc.sync.dma_start(wk8h[:], wk8_d[:, h])
                return wq16h, wk16h, wq8h, wk8h

            def full_proj(wq16h, wk16h, wq8h, wk8h, first=False):
                qraw = w1.tile([P, S], F16, tag="qraw")
                kraw = w1.tile([P, S], F16, tag="kraw")
                proj_corner(wq16h, qraw)
                proj_corner(wk16h, kraw)
                if first:
                    for ti in range(4):
                        v_corner(ti)
                for c in range(1, 4):
                    proj_fp8(wq8h, qraw, c)
                    proj_fp8(wk8h, kraw, c)
                    if first:
                        for ti in range(4 * c, 4 * c + 4):
                            v_fp8(ti)
                return rope(qraw, "qrot"), rope(kraw, "krot")

            rots = {0: full_proj(wq16_h, wk16_h, wq8_h, wk8_h, first=True)}

            def emit_f16(qi, pool):
                for nn in range(4):
                    ps = pool.tile([P, 512], F32, tag="pj")
                    for hh in range(HPC):
                        nc.tensor.matmul(ps[:], OT16[:, hh, ts(qi, P)],
                                         wo16[:, hh, ts(nn, 512)],
                                         start=(hh == 0), stop=(hh == HPC - 1))
                    f_sb = fop.tile([P, 512], F16, tag="fsb")
                    nc.vector.tensor_copy(f_sb[:], ps[:])
                    nc.sync.dma_start(out_d[:, qi, ts(nn, 512)], f_sb[:])

            def emit_f8(qi, pool):
                # qi 1-3 read natural-scale OT8 (qc0 region), rest carry x32
                dsc = 1.0 / WS if qi < 4 else 1.0 / (WS * OS)
                for nn in range(4):
                    ps = pool.tile([P, 512], F32, tag="pj")
                    for j in range(HPC // 2):
                        nc.tensor.matmul(ps[:], OT8[:, 2 * j:2 * j + 2,
                                                     ts(qi, P)],
                                         wo8[:, 2 * j:2 * j + 2, ts(nn, 512)],
                                         start=(j == 0),
                                         stop=(j == HPC // 2 - 1),
                                         perf_mode=DR)
                    f_sb = fop.tile([P, 512], F16, tag="fsb")
                    nc.vector.tensor_scalar_mul(f_sb[:], ps[:], dsc)
                    nc.sync.dma_start(out_d[:, qi, ts(nn, 512)], f_sb[:])

            # ---- attention ----------------------------------------------
            # Per (h, qc): PV/denominator matmuls run one exp-pair behind the
            # score matmuls so the PE never waits on the ACT exp; next head's
            # projection chains are popped one per pair as PE filler work.
            with tc.tile_pool(name="sp", bufs=2, space="PSUM") as stp, \
                 tc.tile_pool(name="op", bufs=1, space="PSUM") as opp, \
                 tc.tile_pool(name="rp", bufs=1, space="PSUM") as rpp:
                for h in range(HPC):
                    qrot, krot = rots.pop(h)
                    chains = []
                    for qc in range(NQ):
                        o_ps = opp.tile([P, 512], F32, tag="o")
                        r_ps = rpp.tile([P, 512], F32, tag="r")
                        qlo, qhi = 512 * qc, 512 * (qc + 1)
                        fq = (list(range(4 * (qc - 1), 4 * qc))
                              if h == HPC - 1 and qc > 0 else [])

                        # pending (kind, kt0, off, e-tile) awaiting PV/denom
                        pend = [None]

                        def flush(last):
                            kind, kt0p, offp, ep = pend[0]
                            if kind == "corner":
                                for i in range(2):
                                    kt = kt0p + i
                                    off = 128 * kt
                                    nc.tensor.matmul(
                                        o_ps[:, off:], V16[:, kt, ts(h, HD)],
                                        ep[:, i, off:],
                                        start=(kt == 0),
                                        stop=(last and i == 1))
                                    nc.tensor.matmul(
                                        r_ps[:, off:], ones16[:],
                                        ep[:, i, off:],
                                        start=(kt == 0),
                                        stop=(last and i == 1))
                            else:
                                nc.tensor.matmul(
                                    o_ps[:, offp:],
                                    V8[:, kt0p // 2, :, ts(h, HD)],
                                    ep[:, :, offp:], start=(kt0p == 0),
                                    stop=last, perf_mode=DR)
                                nc.tensor.matmul(
                                    r_ps[:, offp:], oneq[:], ep[:, :, offp:],
                                    start=(kt0p == 0), stop=last,
                                    perf_mode=DR)
                            pend[0] = None

                        if qc == 0:
                            pairs = [("corner", 2 * j2, 0) for j2 in range(2)]
                        else:
                            pairs = ([("full", 2 * j, 0)
                                      for j in range(2 * qc)] +
                                     [("band", 4 * qc + 2 * j2, 256 * j2)
                                      for j2 in range(2)])
                        for kind, kt0, off in pairs:
                            sp = stp.tile([P, 2, 512], F32, tag="pair")
                            if kind == "corner":
                                for i in range(2):
                                    kt = kt0 + i
                                    o = 128 * kt
                                    nc.tensor.matmul(
                                        sp[:, i, o:], krot[:, ts(kt, P)],
                                        qrot[:, o:512], start=True, stop=False)
                                    nc.tensor.matmul(
                                        sp[:, i, o:o + 128], eye16[:],
                                        mtri16[:], start=False, stop=True)
                                off = 128 * kt0
                            elif kind == "full":
                                for i in range(2):
                                    nc.tensor.matmul(
                                        sp[:, i, :], krot[:, ts(kt0 + i, P)],
                                        qrot[:, qlo:qhi],
                                        start=True, stop=True)
                            else:
                                nc.tensor.matmul(
                                    sp[:, 0, off:], krot[:, ts(kt0, P)],
                                    qrot[:, qlo + off:qhi],
                                    start=True, stop=False)
                                nc.tensor.matmul(
                                    sp[:, 0, off:off + 128], eye16[:],
                                    mtri16[:], start=False, stop=True)
                                nc.tensor.matmul(
                                    sp[:, 1, off:], krot[:, ts(kt0 + 1, P)],
                                    qrot[:, qlo + off:qhi],
                                    start=True, stop=False)
                                nc.tensor.matmul(
                                    sp[:, 1, off:off + 256], eye16[:],
                                    mtri2[:], start=False, stop=True)
                            if kind == "corner":
                                et = etp.tile([P, 2, 512], F16, tag="e16")
                                nc.scalar.activation(et[:, :, off:],
                                                     sp[:, :, off:], EXP,
                                                     scale=SCALE,
                                                     bias=bias4[:])
                            else:
                                et = etp.tile([P, 2, 512], F8, tag="e2")
                                nc.scalar.activation(et[:, :, off:],
                                                     sp[:, :, off:], EXP,
                                                     scale=SCALE,
                                                     bias=bias2[:])
                            if pend[0] is not None:
                                flush(False)
                            pend[0] = (kind, kt0, off, et)
                            if fq:
                                (emit_f16 if fq[0] < 1 else emit_f8)(
                                    fq.pop(0), pjp)
                            elif chains:
                                chains.pop(0)()
                        flush(True)
                        # evict o_ps to SBUF promptly (ACT) so the PSUM bank
                        # frees without waiting on the rinv->normalize chain;
                        # normalize then runs 2x-mode from SBUF off-path
                        o_sb = fop.tile([P, 512], F16, tag="oraw")
                        nc.scalar.copy(o_sb[:], o_ps[:])
                        rinv = rvp.tile([P, 512], F32, tag="rinv")
                        nc.vector.reciprocal_approx_fast(out=rinv[:],
                                                         in_=r_ps[:])
                        if qc == 0:
                            nc.vector.tensor_mul(OT16[:, h, :], o_sb[:],
                                                 rinv[:])
                            # natural-scale fp8 copy of tokens 128-511 for
                            # the fp8 output projection of qi 1-3
                            nc.vector.tensor_mul(OT8[:, h, 128:512],
                                                 o_sb[:, 128:512],
                                                 rinv[:, 128:512])
                        else:
                            nc.vector.tensor_mul(OT8[:, h, ts(qc, 512)],
                                                 o_sb[:], rinv[:])
                        if h + 1 < HPC and qc == 0:
                            # corner proj bulk here; fp8 chains + rope are
                            # deferred into the qc>=1 pair loops as PE filler
                            wq16n, wk16n, wq8n, wk8n = load_w(h + 1)
                            qraw_n = w1.tile([P, S], F16, tag="qraw")
                            kraw_n = w1.tile([P, S], F16, tag="kraw")
                            proj_corner(wq16n, qraw_n)
                            proj_corner(wk16n, kraw_n)
                            for c in range(1, 4):
                                chains.append(
                                    lambda c=c: proj_fp8(wq8n, qraw_n, c))
                                chains.append(
                                    lambda c=c: proj_fp8(wk8n, kraw_n, c))

                            def do_rope(hn=h + 1, q=qraw_n, k=kraw_n):
                                rots[hn] = (rope(q, "qrot"),
                                            rope(k, "krot"))
                            chains.append(do_rope)
                    if h == HPC - 1:
                        for qi in range(12, 16):
                            emit_f8(qi, pjp)

    nc.compile()
    return nc


def _prep_inputs_causal8(x, wq, wk, wv, wo, freqs_cos, freqs_sin):
    """Build the 8 per-core input maps for the fp8 causal kernel."""
    perm = np.concatenate([np.arange(0, HD, 2), np.arange(1, HD, 2)])

    cosT = freqs_cos.T.astype(np.float32)
    sinT = freqs_sin.T.astype(np.float32)
    c2 = np.concatenate([cosT, cosT], 0).astype(NPF16)
    s2n = np.concatenate([-sinT, sinT], 0).astype(NPF16)

    pp, ff = np.meshgrid(np.arange(P), np.arange(P), indexing="ij")
    mtri = np.where(pp > ff, MASK_NEG, 0.0).astype(NPF16)
    pp2, ff2 = np.meshgrid(np.arange(P), np.arange(256), indexing="ij")
    mtri2 = np.where(ff2 < pp2 + 128, MASK_NEG, 0.0).astype(NPF16)

    common = {
        "c2": c2, "s2n": s2n,
        "eye": np.eye(P, dtype=NPF16),
        "mtri": mtri, "mtri2": mtri2,
        "ones": np.ones((P, P), NPF16),
        "oneq": np.full((P, 2, P), 1.0 / OS, NPF8),
    }

    xT_by_b = []
    for b in range(B):
        xT = np.ascontiguousarray(
            x[b].T.reshape(KC, P, NQ, 512).transpose(1, 2, 0, 3))
        xT_by_b.append((np.ascontiguousarray(xT[:, 0]).astype(NPF16),
                        np.ascontiguousarray(xT[:, 1:]).astype(NPF8)))

    in_maps = []
    for c in range(NCORES):
        b, g = divmod(c, TP)
        heads = range(g * HPC, (g + 1) * HPC)
        cols_qk = np.concatenate([h * HD + perm for h in heads])
        cols_v = np.concatenate([np.arange(h * HD, (h + 1) * HD)
                                 for h in heads])

        wq_c = np.ascontiguousarray(
            wq[:, cols_qk].reshape(KC, P, HPC, HD).transpose(1, 2, 0, 3))
        wk_c = np.ascontiguousarray(
            wk[:, cols_qk].reshape(KC, P, HPC, HD).transpose(1, 2, 0, 3))
        wv_c = np.ascontiguousarray(
            wv[:, cols_v].reshape(KC, P, DVC).transpose(1, 0, 2))
        wo_c = np.ascontiguousarray(
            wo[cols_v, :].reshape(HPC, P, D).transpose(1, 0, 2))

        im = dict(common)
        im["xT16"], im["xT8"] = xT_by_b[b]
        im["wq16"] = wq_c.astype(NPF16)
        im["wk16"] = wk_c.astype(NPF16)
        im["wq8"] = (wq_c * WS).astype(NPF8)
        im["wk8"] = (wk_c * WS).astype(NPF8)
        im["wv16"] = wv_c.astype(NPF16)
        im["wv8"] = (wv_c * WS).astype(NPF8)
        im["wo16"] = wo_c.astype(NPF16)
        im["wo8"] = (wo_c * WS).astype(NPF8)
        in_maps.append(im)
    return in_maps


def _build(mask_mode: str):
    """Build + compile the SPMD program. mask_mode: 'causal'|'none'|'general'."""
    nc = bacc.Bacc("TRN2", target_bir_lowering=False, debug=False,
                   num_devices=NCORES)

    def din(name, shape, dt=F16):
        return nc.dram_tensor(name, shape, dt, kind="ExternalInput").ap()

    xT_d = din("xT", [P, NQ, KC, 512])
    wq_d = din("wq", [P, HPC, KC, HD])
    wk_d = din("wk", [P, HPC, KC, HD])
    wv_d = din("wv", [P, KC, DVC])
    wo_d = din("wo", [P, HPC, D])
    c2_d = din("c2", [P, S])
    s2n_d = din("s2n", [P, S])
    ones_d = din("ones", [P, P])
    if mask_mode == "causal":
        eye_d = din("eye", [P, P])
        mtri_d = din("mtri", [P, P])
    elif mask_mode == "general":
        msk_d = din("expm", [P, NT, S])
    out_d = nc.dram_tensor("out", [P, NT, D], mybir.dt.float16,
                           kind="ExternalOutput").ap()

    with tile.TileContext(nc) as tc:
        with tc.tile_pool(name="static", bufs=1) as st, \
             tc.tile_pool(name="w1", bufs=1) as w1, \
             tc.tile_pool(name="w2", bufs=2) as w2, \
             tc.tile_pool(name="et", bufs=6) as etp, \
             tc.tile_pool(name="ac", bufs=3) as accp, \
             tc.tile_pool(name="fo", bufs=4) as fop, \
             tc.tile_pool(name="pj", bufs=2, space="PSUM") as pjp:

            # ---- static tensors -------------------------------------------
            xT = st.tile([P, NQ, KC, 512], F16, tag="xT")
            wv_sb = st.tile([P, KC, DVC], F16, tag="wv")
            wo_sb = st.tile([P, HPC, D], F16, tag="wo")
            c2 = st.tile([P, S], F16, tag="c2")
            s2n = st.tile([P, S], F16, tag="s2n")
            ones_sb = st.tile([P, P], F16, tag="ones")
            V_sb = st.tile([P, NT, DVC], F16, tag="V")
            OT_sb = st.tile([P, HPC, S], F16, tag="OT")
            if mask_mode == "causal":
                eye_sb = st.tile([P, P], F16, tag="eye")
                mtri_sb = st.tile([P, P], F16, tag="mtri")
            bias4 = st.tile([P, 1], F32, tag="b4")
            nc.vector.memset(bias4[:], -4.0)

            # head-0 weights first (small), then interleaved wv/xT chunks so
            # the V-phase matmuls can start as soon as chunk 0 lands.
            wq_h = w1.tile([P, KC, HD], F16, tag="wqh")
            wk_h = w1.tile([P, KC, HD], F16, tag="wkh")
            for g in range(4):
                nc.sync.dma_start(wq_h[:, ts(g, 4), :], wq_d[:, 0, ts(g, 4), :])
                nc.sync.dma_start(xT[:, 0, ts(g, 4), :], xT_d[:, 0, ts(g, 4), :])
            nc.sync.dma_start(wk_h[:], wk_d[:, 0])
            nc.sync.dma_start(wv_sb[:], wv_d)
            nc.sync.dma_start(xT[:, 1, :, :], xT_d[:, 1, :, :])
            nc.sync.dma_start(xT[:, 2, :, :], xT_d[:, 2, :, :])
            nc.sync.dma_start(xT[:, 3, :, :], xT_d[:, 3, :, :])
            nc.sync.dma_start(c2[:], c2_d)
            nc.sync.dma_start(s2n[:], s2n_d)
            nc.sync.dma_start(ones_sb[:], ones_d)
            if mask_mode == "causal":
                nc.sync.dma_start(eye_sb[:], eye_d)
                nc.sync.dma_start(mtri_sb[:], mtri_d)
            nc.sync.dma_start(wo_sb[:], wo_d)

            def proj_half(w_h, raw_tag, rot_tag):
                """One projection (Q or K) + RoPE -> rotated [HD, S] tile."""
                raw = w1.tile([P, S], F16, tag=raw_tag)
                for t in range(NQ):
                    ps = pjp.tile([P, 512], F32, tag="pj")
                    for kc in range(KC):
                        nc.tensor.matmul(ps[:], w_h[:, kc, :],
                                         xT[:, t, kc, :],
                                         start=(kc == 0), stop=(kc == KC - 1))
                    nc.scalar.copy(raw[:, ts(t, 512)], ps[:])
                return rope(raw, rot_tag)

            def load_w(h):
                wq_h = w1.tile([P, KC, HD], F16, tag="wqh")
                nc.sync.dma_start(wq_h[:], wq_d[:, h])
                wk_h = w1.tile([P, KC, HD], F16, tag="wkh")
                nc.sync.dma_start(wk_h[:], wk_d[:, h])
                return wq_h, wk_h

            # head-0 projections first, Q/K interleaved per token chunk so the
            # PE consumes xT chunks as the startup DMAs land; the RoPE chains
            # then run during the V phase.
            def rope(raw, rot_tag):
                swp = w1.tile([P, S], F16, tag="swap")
                nc.vector.tensor_copy(swp[0:64, :], raw[64:128, :])
                nc.vector.tensor_copy(swp[64:128, :], raw[0:64, :])
                rot = w2.tile([P, S], F16, tag=rot_tag)
                nc.vector.tensor_mul(rot[:], raw[:], c2[:])
                nc.vector.tensor_mul(swp[:], swp[:], s2n[:])
                nc.vector.tensor_add(rot[:], rot[:], swp[:])
                return rot

            qraw0 = w1.tile([P, S], F16, tag="qraw")
            kraw0 = w1.tile([P, S], F16, tag="kraw")
            for t in range(NQ):
                for w_h, raw in ((wq_h, qraw0), (wk_h, kraw0)):
                    ps = pjp.tile([P, 512], F32, tag="pj")
                    for kc in range(KC):
                        nc.tensor.matmul(ps[:], w_h[:, kc, :], xT[:, t, kc, :],
                                         start=(kc == 0), stop=(kc == KC - 1))
                    nc.scalar.copy(raw[:, ts(t, 512)], ps[:])
                # V projection for this token chunk keeps the PE busy while
                # the next xT chunk is still streaming in
                for ti in range(4 * t, 4 * t + 4):
                    ps = pjp.tile([P, 512], F32, tag="pj")
                    for kc in range(KC):
                        nc.tensor.matmul(ps[:], xT[:, t, kc, ts(ti % 4, P)],
                                         wv_sb[:, kc, :],
                                         start=(kc == 0), stop=(kc == KC - 1))
                    nc.scalar.copy(V_sb[:, ti, :], ps[:])
            rots = {0: (rope(qraw0, "qrot"), rope(kraw0, "krot"))}

            flip = [False]

            def emit_f(qi, pool, scalar_only=False):
                for nn in range(D // 512):
                    ps = pool.tile([P, 512], F32, tag="pj")
                    for hh in range(HPC):
                        nc.tensor.matmul(ps[:], OT_sb[:, hh, ts(qi, P)],
                                         wo_sb[:, hh, ts(nn, 512)],
                                         start=(hh == 0), stop=(hh == HPC - 1))
                    f_sb = fop.tile([P, 512], F16, tag="fsb")
                    # keep the copies off the Vector engine when F is inlined
                    # into attention: they would head-of-line-block the acc
                    # adds in DVE's in-order queue
                    if flip[0] and not scalar_only:
                        nc.vector.tensor_copy(f_sb[:], ps[:])
                    else:
                        nc.scalar.copy(f_sb[:], ps[:])
                    flip[0] = not flip[0]
                    nc.sync.dma_start(out_d[:, qi, ts(nn, 512)], f_sb[:])

            # ---- attention, with next head's projections interleaved ------
            with tc.tile_pool(name="sp", bufs=3, space="PSUM") as stp, \
                 tc.tile_pool(name="op", bufs=2, space="PSUM") as opp, \
                 tc.tile_pool(name="rp", bufs=1, space="PSUM") as rpp:
                for h in range(HPC):
                    qrot, krot = rots.pop(h)
                    for qc in range(NQ):
                        o_ps = opp.tile([P, 512], F32, tag="o")
                        r_ps = rpp.tile([P, 512], F32, tag="r")
                        nkt = 4 * (qc + 1) if mask_mode == "causal" else NT
                        nfull = 4 * qc if mask_mode == "causal" else 0
                        # previous chunk's output-projection tiles, spread one
                        # per kt iteration so their PSUM->SBUF copies pace
                        # evenly through both engines' queues
                        fq = (list(range(4 * (qc - 1), 4 * qc))
                              if h == HPC - 1 and mask_mode == "causal" and qc > 0
                              else [])
                        # full-width tiles accumulate on the DVE into `acc`;
                        # one ones-matmul on the sum replaces one per tile.
                        acc = first_e = None
                        for kt in range(nkt):
                            band = mask_mode == "causal" and kt >= nfull
                            off = 128 * (kt - nfull) if band else 0
                            s_ps = stp.tile([P, 512], F32, tag="s")
                            nc.tensor.matmul(
                                s_ps[:, off:], krot[:, ts(kt, P)],
                                qrot[:, 512 * qc + off: 512 * (qc + 1)],
                                start=True, stop=not band)
                            if band:
                                nc.tensor.matmul(
                                    s_ps[:, off:off + 128], eye_sb[:], mtri_sb[:],
                                    start=False, stop=True)
                            eT = etp.tile([P, 512], F16, tag="e")
                            # bias -4 (softmax is shift-invariant; the ones-
                            # matmul denominator absorbs it) keeps exp outputs
                            # well inside fp16 range even for hot scores
                            nc.scalar.activation(eT[:, off:], s_ps[:, off:], EXP,
                                                 scale=SCALE, bias=bias4[:])
                            if mask_mode == "general":
                                em = etp.tile([P, 512], F16, tag="em")
                                nc.sync.dma_start(em[:], msk_d[:, kt, ts(qc, 512)])
                                nc.gpsimd.tensor_mul(eT[:], eT[:], em[:])
                            nc.tensor.matmul(o_ps[:, off:],
                                             V_sb[:, kt, ts(h, HD)], eT[:, off:],
                                             start=(kt == 0), stop=(kt == nkt - 1))
                            if mask_mode != "causal":
                                nc.tensor.matmul(r_ps[:], ones_sb[:], eT[:],
                                                 start=(kt == 0),
                                                 stop=(kt == nkt - 1))
                            elif not band:
                                if first_e is not None:
                                    acc = accp.tile([P, 512], F16, tag="acc")
                                    nc.vector.tensor_add(acc[:], first_e[:], eT[:])
                                    first_e = None
                                elif acc is not None:
                                    nc.vector.tensor_add(acc[:], acc[:], eT[:])
                                else:
                                    first_e = eT
                            elif acc is None and first_e is None:
                                acc = accp.tile([P, 512], F16, tag="acc")
                                nc.vector.tensor_copy(acc[:], eT[:])
                            elif first_e is not None:
                                acc = accp.tile([P, 512], F16, tag="acc")
                                nc.vector.tensor_add(acc[:], first_e[:], eT[:])
                                first_e = None
                            else:
                                nc.vector.tensor_add(acc[:, off:], acc[:, off:],
                                                     eT[:, off:])
                            if fq and kt % 3 == 2:
                                emit_f(fq.pop(0), pjp)
                        # leftover output-projection tiles go before the
                        # denominator matmul: they keep the PE busy while the
                        # Scalar engine finishes the trailing band exps
                        for qi in fq:
                            emit_f(qi, pjp)
                        if acc is not None:
                            nc.tensor.matmul(r_ps[:], ones_sb[:], acc[:],
                                             start=True, stop=True)
                        rinv = fop.tile([P, 512], F32, tag="rinv")
                        nc.vector.reciprocal_approx_fast(out=rinv[:], in_=r_ps[:])
                        nc.vector.tensor_mul(OT_sb[:, h, ts(qc, 512)], o_ps[:],
                                             rinv[:])
                        # pipeline the next head's projections + RoPE
                        if h + 1 < HPC and qc == 0:
                            wq_n, wk_n = load_w(h + 1)
                            rots[h + 1] = (proj_half(wq_n, "qraw", "qrot"),
                                           proj_half(wk_n, "kraw", "krot"))
                    if h == HPC - 1 and mask_mode == "causal":
                        for qi in range(4 * (NQ - 1), NT):
                            emit_f(qi, pjp)

            # ---- output projection for non-causal modes (causal streams it
            # inside the last head's attention) --------------------------------
            if mask_mode != "causal":
                with tc.tile_pool(name="fp", bufs=6, space="PSUM") as fpp:
                    for qi in range(NT):
                        emit_f(qi, fpp)

    nc.compile()
    return nc


def _get_program(mask_mode: str):
    if mask_mode not in _cache:
        if mask_mode == "causal":
            _cache[mask_mode] = _build_causal8()
        else:
            _cache[mask_mode] = _build(mask_mode)
    return _cache[mask_mode]


def _detect_mask_mode(mask: np.ndarray) -> str:
    m = mask.reshape(S, S)
    iu = np.triu_indices(S, 1)
    upper = m[iu]
    lower_ok = np.max(np.abs(np.tril(m))) == 0.0
    if lower_ok and upper.size and np.all(upper <= -1e8):
        return "causal"
    if np.max(np.abs(m)) == 0.0:
        return "none"
    return "general"


def _prep_inputs(x, wq, wk, wv, wo, freqs_cos, freqs_sin, mask, mask_mode):
    """Build the 8 per-core input maps (host-side sharding + layout)."""
    # within-head even/odd permutation so RoPE pairs land in partition halves
    perm = np.concatenate([np.arange(0, HD, 2), np.arange(1, HD, 2)])

    cosT = freqs_cos.T.astype(np.float32)          # [64, S]
    sinT = freqs_sin.T.astype(np.float32)
    c2 = np.concatenate([cosT, cosT], 0).astype(NPF16)     # [128, S]
    s2n = np.concatenate([-sinT, sinT], 0).astype(NPF16)
    ones = np.ones((P, P), NPF16)

    common = {"c2": c2, "s2n": s2n, "ones": ones}
    if mask_mode == "causal":
        common["eye"] = np.eye(P, dtype=NPF16)
        pp, ff = np.meshgrid(np.arange(P), np.arange(P), indexing="ij")
        common["mtri"] = np.where(pp > ff, MASK_NEG, 0.0).astype(NPF16)
    elif mask_mode == "general":
        m = mask.reshape(S, S).astype(np.float32)
        # eT[kt_tok, qt_tok] is multiplied by exp(SCALE * mask[qt_tok, kt_tok])
        expm = np.exp(SCALE * m.T).astype(NPF16)            # [k_tok, q_tok]
        common["expm"] = np.ascontiguousarray(
            expm.reshape(NT, P, S).transpose(1, 0, 2))

    xT_by_b = []
    for b in range(B):
        xT = np.ascontiguousarray(
            x[b].T.reshape(KC, P, NQ, 512).transpose(1, 2, 0, 3)).astype(NPF16)
        xT_by_b.append(xT)

    in_maps = []
    for c in range(NCORES):
        b, g = divmod(c, TP)
        heads = range(g * HPC, (g + 1) * HPC)
        cols_qk = np.concatenate([h * HD + perm for h in heads])
        cols_v = np.concatenate([np.arange(h * HD, (h + 1) * HD) for h in heads])

        wq_c = wq[:, cols_qk].reshape(KC, P, HPC, HD).transpose(1, 2, 0, 3)
        wk_c = wk[:, cols_qk].reshape(KC, P, HPC, HD).transpose(1, 2, 0, 3)
        wv_c = wv[:, cols_v].reshape(KC, P, DVC).transpose(1, 0, 2)
        wo_c = wo[cols_v, :].reshape(HPC, P, D).transpose(1, 0, 2)

        im = dict(common)
        im["xT"] = xT_by_b[b]
        im["wq"] = np.ascontiguousarray(wq_c).astype(NPF16)
        im["wk"] = np.ascontiguousarray(wk_c).astype(NPF16)
        im["wv"] = np.ascontiguousarray(wv_c).astype(NPF16)
        im["wo"] = np.ascontiguousarray(wo_c).astype(NPF16)
        in_maps.append(im)
    return in_maps


def run(inputs: dict, **spmd_kwargs):
    """Run on hardware; returns (output [B,S,D] fp32, BassKernelResults)."""
    x = np.asarray(inputs["x"], np.float32)
    wq = np.asarray(inputs["wq"], np.float32)
    wk = np.asarray(inputs["wk"], np.float32)
    wv = np.asarray(inputs["wv"], np.float32)
    wo = np.asarray(inputs["wo"], np.float32)
    fc = np.asarray(inputs["freqs_cos"], np.float32)
    fs = np.asarray(inputs["freqs_sin"], np.float32)
    mask = np.asarray(inputs["mask"], np.float32)

    mask_mode = _detect_mask_mode(mask)
    nc = _get_program(mask_mode)
    if mask_mode == "causal":
        in_maps = _prep_inputs_causal8(x, wq, wk, wv, wo, fc, fs)
    else:
        in_maps = _prep_inputs(x, wq, wk, wv, wo, fc, fs, mask, mask_mode)
    res = run_bass_kernel_spmd(nc, in_maps, core_ids=list(range(NCORES)),
                               **spmd_kwargs)

    out = np.zeros((B, S, D), np.float32)
    for c in range(NCORES):
        b = c // TP
        part = res.results[c]["out"].astype(np.float32)   # [P, NT, D]
        out[b] += part.transpose(1, 0, 2).reshape(S, D)
    return out, res


def kernel(**inputs) -> np.ndarray:
    out, _ = run(inputs)
    return out



# revision 21
# speedup vs baseline: 1.1618x; 1.1618x over previous
"""Trainium2 Bass kernel for nn_Attention_62620623176132.

Multi-head causal attention with RoPE (LLaMA-style), B=2, S=2048, D=2048,
H=16 heads of HD=128, fp32 reference.

Sharding (hardcoded): 8 cores = 2-way data parallel over batch x 4-way
tensor parallel over heads (4 heads per core). Each core computes its 4
heads' Q/K/V projections, attention, and a partial output projection
(rows of wo for its heads); the host sums the 4 fp16 partials per batch
in fp32.

Device algorithm (per core; matmuls in fp16 with fp32 PSUM accumulation):
  - x^T kept SBUF-resident; Q^T/K^T computed per head in [HD, S] layout,
    V in [S, dv] layout, so no transposes are ever needed.
  - RoPE via host-side even/odd column permutation of wq/wk: rotation
    pairs land in partition halves; 3 DVE tensor ops + 2 swap copies.
  - Scores computed transposed, sT[kt, qt] = kT . qT, so exp(sT) feeds
    the PV matmul directly as the moving operand. exp is shifted by -4
    (softmax is shift-invariant) to keep fp16 outputs far from overflow.
  - Softmax denominators: exp tiles are summed on the Vector engine and
    one all-ones stationary matmul per query chunk broadcasts the column
    sums to all partitions; the normalization multiply is fused into the
    PSUM->SBUF copy of the attention output.
  - Causality: score tiles above the diagonal are skipped; band tiles
    are restricted to their unmasked columns and the diagonal square
    gets -60000 added in PSUM by one extra matmul (identity x triangle),
    so masking costs no vector-engine work.
  - Projections for head h+1 are emitted inside head h's attention so
    the serial RoPE chain never stalls the PE; the output projection is
    streamed inside the last head's attention the same way.
  - Startup DMAs are ordered/split so the first projection matmuls start
    as soon as the first x^T chunk lands.

Fallback paths keyed off the runtime mask: all-zero mask -> non-causal
kernel; any other mask -> multiplicative exp(mask/sqrt(HD)) tiles
streamed from DRAM (correct for arbitrary masks, slower).
"""

import math

import numpy as np
import ml_dtypes
import concourse.tile as tile
import concourse.mybir as mybir
from concourse import bacc
from concourse.bass import ts
from concourse.bass_utils import run_bass_kernel_spmd

B, S, D, H, HD = 2, 2048, 2048, 16, 128
P = 128
NCORES = 8
TP = 4                # head-parallel groups
HPC = H // TP         # heads per core = 4
DVC = HPC * HD        # 512 v-dims per core
KC = D // P           # 16 contraction chunks
NT = S // P           # 16 token tiles of 128
NQ = S // 512         # 4 query chunks of 512
F8 = mybir.dt.float8e4
F16 = mybir.dt.float16
F32 = mybir.dt.float32
NPF8 = ml_dtypes.float8_e4m3
NPF16 = np.float16
MASK_NEG = -60000.0
SCALE = 1.0 / math.sqrt(HD)
WS = 64.0             # fp8 weight pre-scale (power of 2, exact)
OS = 32.0             # OT8 scale, via oneq = 1/32 denominator operand
EXP = mybir.ActivationFunctionType.Exp
CPY = mybir.ActivationFunctionType.Copy
DR = mybir.MatmulPerfMode.DoubleRow

_cache: dict = {}


def _build_causal8():
    """fp8 causal kernel: query-chunk 0 (tokens 0-511) fully fp16 (outputs
    there reach ~40 sigma and need low relative error); queries 512+ use
    fp8e4 DoubleRow matmuls wherever the contraction doubles (projections,
    PV pairs, denominator pairs, output projection). Scores stay fp16:
    their contraction is HD=128 and DoubleRow cannot help (measured 1.0x).

    fp8 weights are pre-scaled x64 (exact); 1/64 descale is folded into
    PSUM->SBUF copies. OT8 carries x32 via the 1/32 'oneq' denominator
    matmul; the x2048 on fp8 output-projection PSUM descales in the final
    copy."""
    nc = bacc.Bacc("TRN2", target_bir_lowering=False, debug=False,
                   num_devices=NCORES)

    def din(name, shape, dt):
        return nc.dram_tensor(name, shape, dt, kind="ExternalInput").ap()

    xT16_d = din("xT16", [P, KC, 512], F16)
    xT8_d = din("xT8", [P, 3, KC, 512], F8)
    wq16_d = din("wq16", [P, HPC, KC, HD], F16)
    wk16_d = din("wk16", [P, HPC, KC, HD], F16)
    wq8_d = din("wq8", [P, HPC, KC, HD], F8)
    wk8_d = din("wk8", [P, HPC, KC, HD], F8)
    wv16_d = din("wv16", [P, KC, DVC], F16)
    wv8_d = din("wv8", [P, KC, DVC], F8)
    wo16_d = din("wo16", [P, HPC, D], F16)
    wo8_d = din("wo8", [P, HPC, D], F8)
    c2_d = din("c2", [P, S], F16)
    s2n_d = din("s2n", [P, S], F16)
    eye_d = din("eye", [P, P], F16)
    mtri_d = din("mtri", [P, P], F16)
    mtri2_d = din("mtri2", [P, 256], F16)
    ones_d = din("ones", [P, P], F16)
    oneq_d = din("oneq", [P, 2, P], F8)
    out_d = nc.dram_tensor("out", [P, NT, D], F16, kind="ExternalOutput").ap()

    with tile.TileContext(nc) as tc:
        with tc.tile_pool(name="static", bufs=1) as st, \
             tc.tile_pool(name="w1", bufs=1) as w1, \
             tc.tile_pool(name="w2", bufs=2) as w2, \
             tc.tile_pool(name="et", bufs=4) as etp, \
             tc.tile_pool(name="fo", bufs=4) as fop, \
             tc.tile_pool(name="rv", bufs=2) as rvp, \
             tc.tile_pool(name="pj", bufs=2, space="PSUM") as pjp:

            # ---- static tensors ------------------------------------------
            xT16 = st.tile([P, KC, 512], F16, tag="xT16")
            xT8 = st.tile([P, 3, KC, 512], F8, tag="xT8")
            wv16 = st.tile([P, KC, DVC], F16, tag="wv16")
            wv8 = st.tile([P, KC, DVC], F8, tag="wv8")
            wo16 = st.tile([P, HPC, D], F16, tag="wo16")
            wo8 = st.tile([P, HPC, D], F8, tag="wo8")
            c2 = st.tile([P, S], F16, tag="c2")
            s2n = st.tile([P, S], F16, tag="s2n")
            eye16 = st.tile([P, P], F16, tag="eye")
            mtri16 = st.tile([P, P], F16, tag="mtri")
            mtri2 = st.tile([P, 256], F16, tag="mtri2")
            ones16 = st.tile([P, P], F16, tag="ones")
            oneq = st.tile([P, 2, P], F8, tag="oneq")
            V16 = st.tile([P, 4, DVC], F16, tag="V16")
            V8 = st.tile([P, NT // 2, 2, DVC], F8, tag="V8")
            OT16 = st.tile([P, HPC, 512], F16, tag="OT16")
            OT8 = st.tile([P, HPC, S], F8, tag="OT8")
            bias4 = st.tile([P, 1], F32, tag="b4")
            bias2 = st.tile([P, 1], F32, tag="b2")
            nc.vector.memset(bias4[:], -4.0)
            nc.vector.memset(bias2[:], -2.0)

            # startup DMAs: corner-proj operands first, then the rest
            wq16_h = w1.tile([P, KC, HD], F16, tag="wq16h")
            wk16_h = w1.tile([P, KC, HD], F16, tag="wk16h")
            wq8_h = w1.tile([P, KC, HD], F8, tag="wq8h")
            wk8_h = w1.tile([P, KC, HD], F8, tag="wk8h")
            for g in range(4):
                nc.sync.dma_start(wq16_h[:, ts(g, 4), :],
                                  wq16_d[:, 0, ts(g, 4), :])
                nc.sync.dma_start(xT16[:, ts(g, 4), :], xT16_d[:, ts(g, 4), :])
            nc.sync.dma_start(wk16_h[:], wk16_d[:, 0])
            nc.sync.dma_start(wv16[:], wv16_d)
            nc.sync.dma_start(xT8[:], xT8_d)
            nc.sync.dma_start(wq8_h[:], wq8_d[:, 0])
            nc.sync.dma_start(wk8_h[:], wk8_d[:, 0])
            nc.sync.dma_start(wv8[:], wv8_d)
            nc.sync.dma_start(c2[:], c2_d)
            nc.sync.dma_start(s2n[:], s2n_d)
            nc.sync.dma_start(eye16[:], eye_d)
            nc.sync.dma_start(mtri16[:], mtri_d)
            nc.sync.dma_start(mtri2[:], mtri2_d)
            nc.sync.dma_start(ones16[:], ones_d)
            nc.sync.dma_start(oneq[:], oneq_d)
            nc.sync.dma_start(wo16[:], wo16_d)
            nc.sync.dma_start(wo8[:], wo8_d)

            def rope(raw, rot_tag):
                swp = w1.tile([P, S], F16, tag="swap")
                nc.vector.tensor_copy(swp[0:64, :], raw[64:128, :])
                nc.vector.tensor_copy(swp[64:128, :], raw[0:64, :])
                rot = w2.tile([P, S], F16, tag=rot_tag)
                nc.vector.tensor_mul(rot[:], raw[:], c2[:])
                nc.vector.tensor_mul(swp[:], swp[:], s2n[:])
                nc.vector.tensor_add(rot[:], rot[:], swp[:])
                return rot

            def proj_corner(w16h, raw):
                ps = pjp.tile([P, 512], F32, tag="pj")
                for kc in range(KC):
                    nc.tensor.matmul(ps[:], w16h[:, kc, :], xT16[:, kc, :],
                                     start=(kc == 0), stop=(kc == KC - 1))
                nc.scalar.copy(raw[:, 0:512], ps[:])

            def proj_fp8(w8h, raw, c):
                ps = pjp.tile([P, 512], F32, tag="pj")
                for k in range(KC // 2):
                    nc.tensor.matmul(ps[:], w8h[:, 2 * k:2 * k + 2, :],
                                     xT8[:, c - 1, 2 * k:2 * k + 2, :],
                                     start=(k == 0), stop=(k == KC // 2 - 1),
                                     perf_mode=DR)
                nc.vector.tensor_scalar_mul(raw[:, ts(c, 512)], ps[:], 1.0 / WS)

            def v_corner(ti):
                ps = pjp.tile([P, 512], F32, tag="pj")
                for kc in range(KC):
                    nc.tensor.matmul(ps[:], xT16[:, kc, ts(ti, P)],
                                     wv16[:, kc, :],
                                     start=(kc == 0), stop=(kc == KC - 1))
                nc.scalar.copy(V16[:, ti, :], ps[:])
                nc.scalar.copy(V8[:, ti // 2, ti % 2, :], ps[:])

            def v_fp8(ti):
                ps = pjp.tile([P, 512], F32, tag="pj")
                c = ti // 4
                for k in range(KC // 2):
                    nc.tensor.matmul(ps[:], xT8[:, c - 1, 2 * k:2 * k + 2,
                                                 ts(ti % 4, P)],
                                     wv8[:, 2 * k:2 * k + 2, :],
                                     start=(k == 0), stop=(k == KC // 2 - 1),
                                     perf_mode=DR)
                nc.scalar.activation(V8[:, ti // 2, ti % 2, :], ps[:], CPY,
                                     scale=1.0 / WS)

            def load_w(h):
                wq16h = w1.tile([P, KC, HD], F16, tag="wq16h")
                wk16h = w1.tile([P, KC, HD], F16, tag="wk16h")
                wq8h = w1.tile([P, KC, HD], F8, tag="wq8h")
                wk8h = w1.tile([P, KC, HD], F8, tag="wk8h")
                nc.sync.dma_start(wq16h[:], wq16_d[:, h])
                nc.sync.dma_start(wk16h[:], wk16_d[:, h])
                nc.sync.dma_start(wq8h[:], wq8_d[:, h])
                nc.sync.dma_start(wk8h[:], wk8_d[:, h])
                return wq16h, wk16h, wq8h, wk8h

            def full_proj(wq16h, wk16h, wq8h, wk8h, first=False):
                qraw = w1.tile([P, S], F16, tag="qraw")
                kraw = w1.tile([P, S], F16, tag="kraw")
                proj_corner(wq16h, qraw)
                proj_corner(wk16h, kraw)
                if first:
                    for ti in range(4):
                        v_corner(ti)
                for c in range(1, 4):
                    proj_fp8(wq8h, qraw, c)
                    proj_fp8(wk8h, kraw, c)
                    if first:
                        for ti in range(4 * c, 4 * c + 4):
                            v_fp8(ti)
                return rope(qraw, "qrot"), rope(kraw, "krot")

            rots = {0: full_proj(wq16_h, wk16_h, wq8_h, wk8_h, first=True)}

            def emit_f16(qi, pool):
                for nn in range(4):
                    ps = pool.tile([P, 512], F32, tag="pj")
                    for hh in range(HPC):
                        nc.tensor.matmul(ps[:], OT16[:, hh, ts(qi, P)],
                                         wo16[:, hh, ts(nn, 512)],
                                         start=(hh == 0), stop=(hh == HPC - 1))
                    f_sb = fop.tile([P, 512], F16, tag="fsb")
                    nc.vector.tensor_copy(f_sb[:], ps[:])
                    nc.sync.dma_start(out_d[:, qi, ts(nn, 512)], f_sb[:])

            def emit_f8(qi, pool):
                # qi 1-3 read natural-scale OT8 (qc0 region), rest carry x32
                dsc = 1.0 / WS if qi < 4 else 1.0 / (WS * OS)
                for nn in range(4):
                    ps = pool.tile([P, 512], F32, tag="pj")
                    for j in range(HPC // 2):
                        nc.tensor.matmul(ps[:], OT8[:, 2 * j:2 * j + 2,
                                                     ts(qi, P)],
                                         wo8[:, 2 * j:2 * j + 2, ts(nn, 512)],
                                         start=(j == 0),
                                         stop=(j == HPC // 2 - 1),
                                         perf_mode=DR)
                    f_sb = fop.tile([P, 512], F16, tag="fsb")
                    nc.vector.tensor_scalar_mul(f_sb[:], ps[:], dsc)
                    nc.sync.dma_start(out_d[:, qi, ts(nn, 512)], f_sb[:])

            # ---- attention ----------------------------------------------
            # Per (h, qc): PV/denominator matmuls run one exp-pair behind the
            # score matmuls so the PE never waits on the ACT exp; next head's
            # projection chains are popped one per pair as PE filler work.
            with tc.tile_pool(name="sp", bufs=2, space="PSUM") as stp, \
                 tc.tile_pool(name="op", bufs=1, space="PSUM") as opp, \
                 tc.tile_pool(name="rp", bufs=1, space="PSUM") as rpp:
                for h in range(HPC):
                    qrot, krot = rots.pop(h)
                    for qc in range(NQ):
                        o_ps = opp.tile([P, 512], F32, tag="o")
                        r_ps = rpp.tile([P, 512], F32, tag="r")
                        qlo, qhi = 512 * qc, 512 * (qc + 1)
                        fq = (list(range(4 * (qc - 1), 4 * qc))
                              if h == HPC - 1 and qc > 0 else [])

                        # pending (kind, kt0, off, e-tile) awaiting PV/denom
                        pend = [None]

                        def flush(last):
                            kind, kt0p, offp, ep = pend[0]
                            if kind == "corner":
                                for i in range(2):
                                    kt = kt0p + i
                                    off = 128 * kt
                                    nc.tensor.matmul(
                                        o_ps[:, off:], V16[:, kt, ts(h, HD)],
                                        ep[:, i, off:],
                                        start=(kt == 0),
                                        stop=(last and i == 1))
                                    nc.tensor.matmul(
                                        r_ps[:, off:], ones16[:],
                                        ep[:, i, off:],
                                        start=(kt == 0),
                                        stop=(last and i == 1))
                            else:
                                nc.tensor.matmul(
                                    o_ps[:, offp:],
                                    V8[:, kt0p // 2, :, ts(h, HD)],
                                    ep[:, :, offp:], start=(kt0p == 0),
                                    stop=last, perf_mode=DR)
                                nc.tensor.matmul(
                                    r_ps[:, offp:], oneq[:], ep[:, :, offp:],
                                    start=(kt0p == 0), stop=last,
                                    perf_mode=DR)
                            pend[0] = None

                        if qc == 0:
                            pairs = [("corner", 2 * j2, 0) for j2 in range(2)]
                        else:
                            pairs = ([("full", 2 * j, 0)
                                      for j in range(2 * qc)] +
                                     [("band", 4 * qc + 2 * j2, 256 * j2)
                                      for j2 in range(2)])
                        for kind, kt0, off in pairs:
                            sp = stp.tile([P, 2, 512], F32, tag="pair")
                            if kind == "corner":
                                for i in range(2):
                                    kt = kt0 + i
                                    o = 128 * kt
                                    nc.tensor.matmul(
                                        sp[:, i, o:], krot[:, ts(kt, P)],
                                        qrot[:, o:512], start=True, stop=False)
                                    nc.tensor.matmul(
                                        sp[:, i, o:o + 128], eye16[:],
                                        mtri16[:], start=False, stop=True)
                                off = 128 * kt0
                            elif kind == "full":
                                for i in range(2):
                                    nc.tensor.matmul(
                                        sp[:, i, :], krot[:, ts(kt0 + i, P)],
                                        qrot[:, qlo:qhi],
                                        start=True, stop=True)
                            else:
                                nc.tensor.matmul(
                                    sp[:, 0, off:], krot[:, ts(kt0, P)],
                                    qrot[:, qlo + off:qhi],
                                    start=True, stop=False)
                                nc.tensor.matmul(
                                    sp[:, 0, off:off + 128], eye16[:],
                                    mtri16[:], start=False, stop=True)
                                nc.tensor.matmul(
                                    sp[:, 1, off:], krot[:, ts(kt0 + 1, P)],
                                    qrot[:, qlo + off:qhi],
                                    start=True, stop=False)
                                nc.tensor.matmul(
                                    sp[:, 1, off:off + 256], eye16[:],
                                    mtri2[:], start=False, stop=True)
                            if kind == "corner":
                                et = etp.tile([P, 2, 512], F16, tag="e16")
                                nc.scalar.activation(et[:, :, off:],
                                                     sp[:, :, off:], EXP,
                                                     scale=SCALE,
                                                     bias=bias4[:])
                            else:
                                et = etp.tile([P, 2, 512], F8, tag="e2")
                                nc.scalar.activation(et[:, :, off:],
                                                     sp[:, :, off:], EXP,
                                                     scale=SCALE,
                                                     bias=bias2[:])
                            if pend[0] is not None:
                                flush(False)
                            pend[0] = (kind, kt0, off, et)
                            if fq:
                                (emit_f16 if fq[0] < 1 else emit_f8)(
                                    fq.pop(0), pjp)
                        flush(True)
                        # evict o_ps to SBUF promptly (ACT) so the PSUM bank
                        # frees without waiting on the rinv->normalize chain;
                        # normalize then runs 2x-mode from SBUF off-path
                        o_sb = fop.tile([P, 512], F16, tag="oraw")
                        nc.scalar.copy(o_sb[:], o_ps[:])
                        rinv = rvp.tile([P, 512], F32, tag="rinv")
                        nc.vector.reciprocal_approx_fast(out=rinv[:],
                                                         in_=r_ps[:])
                        if qc == 0:
                            nc.vector.tensor_mul(OT16[:, h, :], o_sb[:],
                                                 rinv[:])
                            # natural-scale fp8 copy of tokens 128-511 for
                            # the fp8 output projection of qi 1-3
                            nc.vector.tensor_mul(OT8[:, h, 128:512],
                                                 o_sb[:, 128:512],
                                                 rinv[:, 128:512])
                        else:
                            nc.vector.tensor_mul(OT8[:, h, ts(qc, 512)],
                                                 o_sb[:], rinv[:])
                        if h + 1 < HPC and qc == 0:
                            rots[h + 1] = full_proj(*load_w(h + 1))
                    if h == HPC - 1:
                        for qi in range(12, 16):
                            emit_f8(qi, pjp)

    nc.compile()
    return nc


def _prep_inputs_causal8(x, wq, wk, wv, wo, freqs_cos, freqs_sin):
    """Build the 8 per-core input maps for the fp8 causal kernel."""
    perm = np.concatenate([np.arange(0, HD, 2), np.arange(1, HD, 2)])

    cosT = freqs_cos.T.astype(np.float32)
    sinT = freqs_sin.T.astype(np.float32)
    c2 = np.concatenate([cosT, cosT], 0).astype(NPF16)
    s2n = np.concatenate([-sinT, sinT], 0).astype(NPF16)

    pp, ff = np.meshgrid(np.arange(P), np.arange(P), indexing="ij")
    mtri = np.where(pp > ff, MASK_NEG, 0.0).astype(NPF16)
    pp2, ff2 = np.meshgrid(np.arange(P), np.arange(256), indexing="ij")
    mtri2 = np.where(ff2 < pp2 + 128, MASK_NEG, 0.0).astype(NPF16)

    common = {
        "c2": c2, "s2n": s2n,
        "eye": np.eye(P, dtype=NPF16),
        "mtri": mtri, "mtri2": mtri2,
        "ones": np.ones((P, P), NPF16),
        "oneq": np.full((P, 2, P), 1.0 / OS, NPF8),
    }

    xT_by_b = []
    for b in range(B):
        xT = np.ascontiguousarray(
            x[b].T.reshape(KC, P, NQ, 512).transpose(1, 2, 0, 3))
        xT_by_b.append((np.ascontiguousarray(xT[:, 0]).astype(NPF16),
                        np.ascontiguousarray(xT[:, 1:]).astype(NPF8)))

    in_maps = []
    for c in range(NCORES):
        b, g = divmod(c, TP)
        heads = range(g * HPC, (g + 1) * HPC)
        cols_qk = np.concatenate([h * HD + perm for h in heads])
        cols_v = np.concatenate([np.arange(h * HD, (h + 1) * HD)
                                 for h in heads])

        wq_c = np.ascontiguousarray(
            wq[:, cols_qk].reshape(KC, P, HPC, HD).transpose(1, 2, 0, 3))
        wk_c = np.ascontiguousarray(
            wk[:, cols_qk].reshape(KC, P, HPC, HD).transpose(1, 2, 0, 3))
        wv_c = np.ascontiguousarray(
            wv[:, cols_v].reshape(KC, P, DVC).transpose(1, 0, 2))
        wo_c = np.ascontiguousarray(
            wo[cols_v, :].reshape(HPC, P, D).transpose(1, 0, 2))

        im = dict(common)
        im["xT16"], im["xT8"] = xT_by_b[b]
        im["wq16"] = wq_c.astype(NPF16)
        im["wk16"] = wk_c.astype(NPF16)
        im["wq8"] = (wq_c * WS).astype(NPF8)
        im["wk8"] = (wk_c * WS).astype(NPF8)
        im["wv16"] = wv_c.astype(NPF16)
        im["wv8"] = (wv_c * WS).astype(NPF8)
        im["wo16"] = wo_c.astype(NPF16)
        im["wo8"] = (wo_c * WS).astype(NPF8)
        in_maps.append(im)
    return in_maps


def _build(mask_mode: str):
    """Build + compile the SPMD program. mask_mode: 'causal'|'none'|'general'."""
    nc = bacc.Bacc("TRN2", target_bir_lowering=False, debug=False,
                   num_devices=NCORES)

    def din(name, shape, dt=F16):
        return nc.dram_tensor(name, shape, dt, kind="ExternalInput").ap()

    xT_d = din("xT", [P, NQ, KC, 512])
    wq_d = din("wq", [P, HPC, KC, HD])
    wk_d = din("wk", [P, HPC, KC, HD])
    wv_d = din("wv", [P, KC, DVC])
    wo_d = din("wo", [P, HPC, D])
    c2_d = din("c2", [P, S])
    s2n_d = din("s2n", [P, S])
    ones_d = din("ones", [P, P])
    if mask_mode == "causal":
        eye_d = din("eye", [P, P])
        mtri_d = din("mtri", [P, P])
    elif mask_mode == "general":
        msk_d = din("expm", [P, NT, S])
    out_d = nc.dram_tensor("out", [P, NT, D], mybir.dt.float16,
                           kind="ExternalOutput").ap()

    with tile.TileContext(nc) as tc:
        with tc.tile_pool(name="static", bufs=1) as st, \
             tc.tile_pool(name="w1", bufs=1) as w1, \
             tc.tile_pool(name="w2", bufs=2) as w2, \
             tc.tile_pool(name="et", bufs=6) as etp, \
             tc.tile_pool(name="ac", bufs=3) as accp, \
             tc.tile_pool(name="fo", bufs=4) as fop, \
             tc.tile_pool(name="pj", bufs=2, space="PSUM") as pjp:

            # ---- static tensors -------------------------------------------
            xT = st.tile([P, NQ, KC, 512], F16, tag="xT")
            wv_sb = st.tile([P, KC, DVC], F16, tag="wv")
            wo_sb = st.tile([P, HPC, D], F16, tag="wo")
            c2 = st.tile([P, S], F16, tag="c2")
            s2n = st.tile([P, S], F16, tag="s2n")
            ones_sb = st.tile([P, P], F16, tag="ones")
            V_sb = st.tile([P, NT, DVC], F16, tag="V")
            OT_sb = st.tile([P, HPC, S], F16, tag="OT")
            if mask_mode == "causal":
                eye_sb = st.tile([P, P], F16, tag="eye")
                mtri_sb = st.tile([P, P], F16, tag="mtri")
            bias4 = st.tile([P, 1], F32, tag="b4")
            nc.vector.memset(bias4[:], -4.0)

            # head-0 weights first (small), then interleaved wv/xT chunks so
            # the V-phase matmuls can start as soon as chunk 0 lands.
            wq_h = w1.tile([P, KC, HD], F16, tag="wqh")
            wk_h = w1.tile([P, KC, HD], F16, tag="wkh")
            for g in range(4):
                nc.sync.dma_start(wq_h[:, ts(g, 4), :], wq_d[:, 0, ts(g, 4), :])
                nc.sync.dma_start(xT[:, 0, ts(g, 4), :], xT_d[:, 0, ts(g, 4), :])
            nc.sync.dma_start(wk_h[:], wk_d[:, 0])
            nc.sync.dma_start(wv_sb[:], wv_d)
            nc.sync.dma_start(xT[:, 1, :, :], xT_d[:, 1, :, :])
            nc.sync.dma_start(xT[:, 2, :, :], xT_d[:, 2, :, :])
            nc.sync.dma_start(xT[:, 3, :, :], xT_d[:, 3, :, :])
            nc.sync.dma_start(c2[:], c2_d)
            nc.sync.dma_start(s2n[:], s2n_d)
            nc.sync.dma_start(ones_sb[:], ones_d)
            if mask_mode == "causal":
                nc.sync.dma_start(eye_sb[:], eye_d)
                nc.sync.dma_start(mtri_sb[:], mtri_d)
            nc.sync.dma_start(wo_sb[:], wo_d)

            def proj_half(w_h, raw_tag, rot_tag):
                """One projection (Q or K) + RoPE -> rotated [HD, S] tile."""
                raw = w1.tile([P, S], F16, tag=raw_tag)
                for t in range(NQ):
                    ps = pjp.tile([P, 512], F32, tag="pj")
                    for kc in range(KC):
                        nc.tensor.matmul(ps[:], w_h[:, kc, :],
                                         xT[:, t, kc, :],
                                         start=(kc == 0), stop=(kc == KC - 1))
                    nc.scalar.copy(raw[:, ts(t, 512)], ps[:])
                return rope(raw, rot_tag)

            def load_w(h):
                wq_h = w1.tile([P, KC, HD], F16, tag="wqh")
                nc.sync.dma_start(wq_h[:], wq_d[:, h])
                wk_h = w1.tile([P, KC, HD], F16, tag="wkh")
                nc.sync.dma_start(wk_h[:], wk_d[:, h])
                return wq_h, wk_h

            # head-0 projections first, Q/K interleaved per token chunk so the
            # PE consumes xT chunks as the startup DMAs land; the RoPE chains
            # then run during the V phase.
            def rope(raw, rot_tag):
                swp = w1.tile([P, S], F16, tag="swap")
                nc.vector.tensor_copy(swp[0:64, :], raw[64:128, :])
                nc.vector.tensor_copy(swp[64:128, :], raw[0:64, :])
                rot = w2.tile([P, S], F16, tag=rot_tag)
                nc.vector.tensor_mul(rot[:], raw[:], c2[:])
                nc.vector.tensor_mul(swp[:], swp[:], s2n[:])
                nc.vector.tensor_add(rot[:], rot[:], swp[:])
                return rot

            qraw0 = w1.tile([P, S], F16, tag="qraw")
            kraw0 = w1.tile([P, S], F16, tag="kraw")
            for t in range(NQ):
                for w_h, raw in ((wq_h, qraw0), (wk_h, kraw0)):
                    ps = pjp.tile([P, 512], F32, tag="pj")
                    for kc in range(KC):
                        nc.tensor.matmul(ps[:], w_h[:, kc, :], xT[:, t, kc, :],
                                         start=(kc == 0), stop=(kc == KC - 1))
                    nc.scalar.copy(raw[:, ts(t, 512)], ps[:])
                # V projection for this token chunk keeps the PE busy while
                # the next xT chunk is still streaming in
                for ti in range(4 * t, 4 * t + 4):
                    ps = pjp.tile([P, 512], F32, tag="pj")
                    for kc in range(KC):
                        nc.tensor.matmul(ps[:], xT[:, t, kc, ts(ti % 4, P)],
                                         wv_sb[:, kc, :],
                                         start=(kc == 0), stop=(kc == KC - 1))
                    nc.scalar.copy(V_sb[:, ti, :], ps[:])
            rots = {0: (rope(qraw0, "qrot"), rope(kraw0, "krot"))}

            flip = [False]

            def emit_f(qi, pool, scalar_only=False):
                for nn in range(D // 512):
                    ps = pool.tile([P, 512], F32, tag="pj")
                    for hh in range(HPC):
                        nc.tensor.matmul(ps[:], OT_sb[:, hh, ts(qi, P)],
                                         wo_sb[:, hh, ts(nn, 512)],
                                         start=(hh == 0), stop=(hh == HPC - 1))
                    f_sb = fop.tile([P, 512], F16, tag="fsb")
                    # keep the copies off the Vector engine when F is inlined
                    # into attention: they would head-of-line-block the acc
                    # adds in DVE's in-order queue
                    if flip[0] and not scalar_only:
                        nc.vector.tensor_copy(f_sb[:], ps[:])
                    else:
                        nc.scalar.copy(f_sb[:], ps[:])
                    flip[0] = not flip[0]
                    nc.sync.dma_start(out_d[:, qi, ts(nn, 512)], f_sb[:])

            # ---- attention, with next head's projections interleaved ------
            with tc.tile_pool(name="sp", bufs=3, space="PSUM") as stp, \
                 tc.tile_pool(name="op", bufs=2, space="PSUM") as opp, \
                 tc.tile_pool(name="rp", bufs=1, space="PSUM") as rpp:
                for h in range(HPC):
                    qrot, krot = rots.pop(h)
                    for qc in range(NQ):
                        o_ps = opp.tile([P, 512], F32, tag="o")
                        r_ps = rpp.tile([P, 512], F32, tag="r")
                        nkt = 4 * (qc + 1) if mask_mode == "causal" else NT
                        nfull = 4 * qc if mask_mode == "causal" else 0
                        # previous chunk's output-projection tiles, spread one
                        # per kt iteration so their PSUM->SBUF copies pace
                        # evenly through both engines' queues
                        fq = (list(range(4 * (qc - 1), 4 * qc))
                              if h == HPC - 1 and mask_mode == "causal" and qc > 0
                              else [])
                        # full-width tiles accumulate on the DVE into `acc`;
                        # one ones-matmul on the sum replaces one per tile.
                        acc = first_e = None
                        for kt in range(nkt):
                            band = mask_mode == "causal" and kt >= nfull
                            off = 128 * (kt - nfull) if band else 0
                            s_ps = stp.tile([P, 512], F32, tag="s")
                            nc.tensor.matmul(
                                s_ps[:, off:], krot[:, ts(kt, P)],
                                qrot[:, 512 * qc + off: 512 * (qc + 1)],
                                start=True, stop=not band)
                            if band:
                                nc.tensor.matmul(
                                    s_ps[:, off:off + 128], eye_sb[:], mtri_sb[:],
                                    start=False, stop=True)
                            eT = etp.tile([P, 512], F16, tag="e")
                            # bias -4 (softmax is shift-invariant; the ones-
                            # matmul denominator absorbs it) keeps exp outputs
                            # well inside fp16 range even for hot scores
                            nc.scalar.activation(eT[:, off:], s_ps[:, off:], EXP,
                                                 scale=SCALE, bias=bias4[:])
                            if mask_mode == "general":
                                em = etp.tile([P, 512], F16, tag="em")
                                nc.sync.dma_start(em[:], msk_d[:, kt, ts(qc, 512)])
                                nc.gpsimd.tensor_mul(eT[:], eT[:], em[:])
                            nc.tensor.matmul(o_ps[:, off:],
                                             V_sb[:, kt, ts(h, HD)], eT[:, off:],
                                             start=(kt == 0), stop=(kt == nkt - 1))
                            if mask_mode != "causal":
                                nc.tensor.matmul(r_ps[:], ones_sb[:], eT[:],
                                                 start=(kt == 0),
                                                 stop=(kt == nkt - 1))
                            elif not band:
                                if first_e is not None:
                                    acc = accp.tile([P, 512], F16, tag="acc")
                                    nc.vector.tensor_add(acc[:], first_e[:], eT[:])
                                    first_e = None
                                elif acc is not None:
                                    nc.vector.tensor_add(acc[:], acc[:], eT[:])
                                else:
                                    first_e = eT
                            elif acc is None and first_e is None:
                                acc = accp.tile([P, 512], F16, tag="acc")
                                nc.vector.tensor_copy(acc[:], eT[:])
                            elif first_e is not None:
                                acc = accp.tile([P, 512], F16, tag="acc")
                                nc.vector.tensor_add(acc[:], first_e[:], eT[:])
                                first_e = None
                            else:
                                nc.vector.tensor_add(acc[:, off:], acc[:, off:],
                                                     eT[:, off:])
                            if fq and kt % 3 == 2:
                                emit_f(fq.pop(0), pjp)
                        # leftover output-projection tiles go before the
                        # denominator matmul: they keep the PE busy while the
                        # Scalar engine finishes the trailing band exps
                        for qi in fq:
                            emit_f(qi, pjp)
                        if acc is not None:
                            nc.tensor.matmul(r_ps[:], ones_sb[:], acc[:],
                                             start=True, stop=True)
                        rinv = fop.tile([P, 512], F32, tag="rinv")
                        nc.vector.reciprocal_approx_fast(out=rinv[:], in_=r_ps[:])
                        nc.vector.tensor_mul(OT_sb[:, h, ts(qc, 512)], o_ps[:],
                                             rinv[:])
                        # pipeline the next head's projections + RoPE
                        if h + 1 < HPC and qc == 0:
                            wq_n, wk_n = load_w(h + 1)
                            rots[h + 1] = (proj_half(wq_n, "qraw", "qrot"),
                                           proj_half(wk_n, "kraw", "krot"))
                    if h == HPC - 1 and mask_mode == "causal":
                        for qi in range(4 * (NQ - 1), NT):
                            emit_f(qi, pjp)

            # ---- output projection for non-causal modes (causal streams it
            # inside the last head's attention) --------------------------------
            if mask_mode != "causal":
                with tc.tile_pool(name="fp", bufs=6, space="PSUM") as fpp:
                    for qi in range(NT):
                        emit_f(qi, fpp)

    nc.compile()
    return nc


def _get_program(mask_mode: str):
    if mask_mode not in _cache:
        if mask_mode == "causal":
            _cache[mask_mode] = _build_causal8()
        else:
            _cache[mask_mode] = _build(mask_mode)
    return _cache[mask_mode]


def _detect_mask_mode(mask: np.ndarray) -> str:
    m = mask.reshape(S, S)
    iu = np.triu_indices(S, 1)
    upper = m[iu]
    lower_ok = np.max(np.abs(np.tril(m))) == 0.0
    if lower_ok and upper.size and np.all(upper <= -1e8):
        return "causal"
    if np.max(np.abs(m)) == 0.0:
        return "none"
    return "general"


def _prep_inputs(x, wq, wk, wv, wo, freqs_cos, freqs_sin, mask, mask_mode):
    """Build the 8 per-core input maps (host-side sharding + layout)."""
    # within-head even/odd permutation so RoPE pairs land in partition halves
    perm = np.concatenate([np.arange(0, HD, 2), np.arange(1, HD, 2)])

    cosT = freqs_cos.T.astype(np.float32)          # [64, S]
    sinT = freqs_sin.T.astype(np.float32)
    c2 = np.concatenate([cosT, cosT], 0).astype(NPF16)     # [128, S]
    s2n = np.concatenate([-sinT, sinT], 0).astype(NPF16)
    ones = np.ones((P, P), NPF16)

    common = {"c2": c2, "s2n": s2n, "ones": ones}
    if mask_mode == "causal":
        common["eye"] = np.eye(P, dtype=NPF16)
        pp, ff = np.meshgrid(np.arange(P), np.arange(P), indexing="ij")
        common["mtri"] = np.where(pp > ff, MASK_NEG, 0.0).astype(NPF16)
    elif mask_mode == "general":
        m = mask.reshape(S, S).astype(np.float32)
        # eT[kt_tok, qt_tok] is multiplied by exp(SCALE * mask[qt_tok, kt_tok])
        expm = np.exp(SCALE * m.T).astype(NPF16)            # [k_tok, q_tok]
        common["expm"] = np.ascontiguousarray(
            expm.reshape(NT, P, S).transpose(1, 0, 2))

    xT_by_b = []
    for b in range(B):
        xT = np.ascontiguousarray(
            x[b].T.reshape(KC, P, NQ, 512).transpose(1, 2, 0, 3)).astype(NPF16)
        xT_by_b.append(xT)

    in_maps = []
    for c in range(NCORES):
        b, g = divmod(c, TP)
        heads = range(g * HPC, (g + 1) * HPC)
        cols_qk = np.concatenate([h * HD + perm for h in heads])
        cols_v = np.concatenate([np.arange(h * HD, (h + 1) * HD) for h in heads])

        wq_c = wq[:, cols_qk].reshape(KC, P, HPC, HD).transpose(1, 2, 0, 3)
        wk_c = wk[:, cols_qk].reshape(KC, P, HPC, HD).transpose(1, 2, 0, 3)
        wv_c = wv[:, cols_v].reshape(KC, P, DVC).transpose(1, 0, 2)
        wo_c = wo[cols_v, :].reshape(HPC, P, D).transpose(1, 0, 2)

        im = dict(common)
        im["xT"] = xT_by_b[b]
        im["wq"] = np.ascontiguousarray(wq_c).astype(NPF16)
        im["wk"] = np.ascontiguousarray(wk_c).astype(NPF16)
        im["wv"] = np.ascontiguousarray(wv_c).astype(NPF16)
        im["wo"] = np.ascontiguousarray(wo_c).astype(NPF16)
        in_maps.append(im)
    return in_maps


def run(inputs: dict, **spmd_kwargs):
    """Run on hardware; returns (output [B,S,D] fp32, BassKernelResults)."""
    x = np.asarray(inputs["x"], np.float32)
    wq = np.asarray(inputs["wq"], np.float32)
    wk = np.asarray(inputs["wk"], np.float32)
    wv = np.asarray(inputs["wv"], np.float32)
    wo = np.asarray(inputs["wo"], np.float32)
    fc = np.asarray(inputs["freqs_cos"], np.float32)
    fs = np.asarray(inputs["freqs_sin"], np.float32)
    mask = np.asarray(inputs["mask"], np.float32)

    mask_mode = _detect_mask_mode(mask)
    nc = _get_program(mask_mode)
    if mask_mode == "causal":
        in_maps = _prep_inputs_causal8(x, wq, wk, wv, wo, fc, fs)
    else:
        in_maps = _prep_inputs(x, wq, wk, wv, wo, fc, fs, mask, mask_mode)
    res = run_bass_kernel_spmd(nc, in_maps, core_ids=list(range(NCORES)),
                               **spmd_kwargs)

    out = np.zeros((B, S, D), np.float32)
    for c in range(NCORES):
        b = c // TP
        part = res.results[c]["out"].astype(np.float32)   # [P, NT, D]
        out[b] += part.transpose(1, 0, 2).reshape(S, D)
    return out, res


def kernel(**inputs) -> np.ndarray:
    out, _ = run(inputs)
    return out



# revision 24
# speedup vs baseline: 1.1723x; 1.0091x over previous
"""Trainium2 Bass kernel for nn_Attention_62620623176132.

Multi-head causal attention with RoPE (LLaMA-style), B=2, S=2048, D=2048,
H=16 heads of HD=128, fp32 reference.

Sharding (hardcoded): 8 cores = 2-way data parallel over batch x 4-way
tensor parallel over heads (4 heads per core). Each core computes its 4
heads' Q/K/V projections, attention, and a partial output projection
(rows of wo for its heads); the host sums the 4 fp16 partials per batch
in fp32.

Causal path (_build_causal8): mixed fp16/fp8e4 precision. The causal
output's absmax sits at ~41 sigma on the earliest tokens (they average
few values), so query-chunk 0 (tokens 0-511) is computed fully in fp16;
queries 512+ use fp8e4 DoubleRow matmuls wherever the contraction can be
doubled (QKV projections, PV + denominator key-tile pairs, output
projection). Scores stay fp16 (contraction HD=128; DoubleRow measured
1.0x there). fp8 weights are pre-scaled x64; descales fold into the
PSUM evictions. Measured ~250us vs 327us for the all-fp16 kernel, rel
err 7e-3 vs the 2e-2 gate.

Legacy fp16 algorithm (kept for the non-causal fallback paths):
  - x^T kept SBUF-resident; Q^T/K^T computed per head in [HD, S] layout,
    V in [S, dv] layout, so no transposes are ever needed.
  - RoPE via host-side even/odd column permutation of wq/wk: rotation
    pairs land in partition halves; 3 DVE tensor ops + 2 swap copies.
  - Scores computed transposed, sT[kt, qt] = kT . qT, so exp(sT) feeds
    the PV matmul directly as the moving operand. exp is shifted by -4
    (softmax is shift-invariant) to keep fp16 outputs far from overflow.
  - Softmax denominators: exp tiles are summed on the Vector engine and
    one all-ones stationary matmul per query chunk broadcasts the column
    sums to all partitions; the normalization multiply is fused into the
    PSUM->SBUF copy of the attention output.
  - Causality: score tiles above the diagonal are skipped; band tiles
    are restricted to their unmasked columns and the diagonal square
    gets -60000 added in PSUM by one extra matmul (identity x triangle),
    so masking costs no vector-engine work.
  - Projections for head h+1 are emitted inside head h's attention so
    the serial RoPE chain never stalls the PE; the output projection is
    streamed inside the last head's attention the same way.
  - Startup DMAs are ordered/split so the first projection matmuls start
    as soon as the first x^T chunk lands.

Fallback paths keyed off the runtime mask: all-zero mask -> non-causal
kernel; any other mask -> multiplicative exp(mask/sqrt(HD)) tiles
streamed from DRAM (correct for arbitrary masks, slower).
"""

import math

import numpy as np
import ml_dtypes
import concourse.tile as tile
import concourse.mybir as mybir
from concourse import bacc
from concourse.bass import ts
from concourse.bass_utils import run_bass_kernel_spmd

B, S, D, H, HD = 2, 2048, 2048, 16, 128
P = 128
NCORES = 8
TP = 4                # head-parallel groups
HPC = H // TP         # heads per core = 4
DVC = HPC * HD        # 512 v-dims per core
KC = D // P           # 16 contraction chunks
NT = S // P           # 16 token tiles of 128
NQ = S // 512         # 4 query chunks of 512
F8 = mybir.dt.float8e4
F16 = mybir.dt.float16
F32 = mybir.dt.float32
NPF8 = ml_dtypes.float8_e4m3
NPF16 = np.float16
MASK_NEG = -60000.0
SCALE = 1.0 / math.sqrt(HD)
WS = 64.0             # fp8 weight pre-scale (power of 2, exact)
OS = 32.0             # OT8 scale, via oneq = 1/32 denominator operand
EXP = mybir.ActivationFunctionType.Exp
CPY = mybir.ActivationFunctionType.Copy
DR = mybir.MatmulPerfMode.DoubleRow

_cache: dict = {}


def _build_causal8():
    """fp8 causal kernel: query-chunk 0 (tokens 0-511) fully fp16 (outputs
    there reach ~40 sigma and need low relative error); queries 512+ use
    fp8e4 DoubleRow matmuls wherever the contraction doubles (projections,
    PV pairs, denominator pairs, output projection). Scores stay fp16:
    their contraction is HD=128 and DoubleRow cannot help (measured 1.0x).

    fp8 weights are pre-scaled x64 (exact); 1/64 descale is folded into
    PSUM->SBUF copies. OT8 carries x32 via the 1/32 'oneq' denominator
    matmul; the x2048 on fp8 output-projection PSUM descales in the final
    copy."""
    nc = bacc.Bacc("TRN2", target_bir_lowering=False, debug=False,
                   num_devices=NCORES)

    def din(name, shape, dt):
        return nc.dram_tensor(name, shape, dt, kind="ExternalInput").ap()

    xT16_d = din("xT16", [P, KC, 512], F16)
    xT8_d = din("xT8", [P, 3, KC, 512], F8)
    wq16_d = din("wq16", [P, HPC, KC, HD], F16)
    wk16_d = din("wk16", [P, HPC, KC, HD], F16)
    wq8_d = din("wq8", [P, HPC, KC, HD], F8)
    wk8_d = din("wk8", [P, HPC, KC, HD], F8)
    wv16_d = din("wv16", [P, KC, DVC], F16)
    wv8_d = din("wv8", [P, KC, DVC], F8)
    wo16_d = din("wo16", [P, HPC, D], F16)
    wo8_d = din("wo8", [P, HPC, D], F8)
    c2_d = din("c2", [P, S], F16)
    s2n_d = din("s2n", [P, S], F16)
    eye_d = din("eye", [P, P], F16)
    mtri_d = din("mtri", [P, P], F16)
    mtri2_d = din("mtri2", [P, 256], F16)
    ones_d = din("ones", [P, P], F16)
    oneq_d = din("oneq", [P, 2, P], F8)
    out_d = nc.dram_tensor("out", [P, NT, D], F16, kind="ExternalOutput").ap()

    with tile.TileContext(nc) as tc:
        with tc.tile_pool(name="static", bufs=1) as st, \
             tc.tile_pool(name="w1", bufs=1) as w1, \
             tc.tile_pool(name="w2", bufs=2) as w2, \
             tc.tile_pool(name="et", bufs=4) as etp, \
             tc.tile_pool(name="fo", bufs=4) as fop, \
             tc.tile_pool(name="rv", bufs=2) as rvp, \
             tc.tile_pool(name="pj", bufs=2, space="PSUM") as pjp:

            # ---- static tensors ------------------------------------------
            xT16 = st.tile([P, KC, 512], F16, tag="xT16")
            xT8 = st.tile([P, 3, KC, 512], F8, tag="xT8")
            wv16 = st.tile([P, KC, DVC], F16, tag="wv16")
            wv8 = st.tile([P, KC, DVC], F8, tag="wv8")
            wo16 = st.tile([P, HPC, D], F16, tag="wo16")
            wo8 = st.tile([P, HPC, D], F8, tag="wo8")
            c2 = st.tile([P, S], F16, tag="c2")
            s2n = st.tile([P, S], F16, tag="s2n")
            eye16 = st.tile([P, P], F16, tag="eye")
            mtri16 = st.tile([P, P], F16, tag="mtri")
            mtri2 = st.tile([P, 256], F16, tag="mtri2")
            ones16 = st.tile([P, P], F16, tag="ones")
            oneq = st.tile([P, 2, P], F8, tag="oneq")
            V16 = st.tile([P, 4, DVC], F16, tag="V16")
            V8 = st.tile([P, NT // 2, 2, DVC], F8, tag="V8")
            OT16 = st.tile([P, HPC, 512], F16, tag="OT16")
            OT8 = st.tile([P, HPC, S], F8, tag="OT8")
            bias4 = st.tile([P, 1], F32, tag="b4")
            bias2 = st.tile([P, 1], F32, tag="b2")
            nc.vector.memset(bias4[:], -4.0)
            nc.vector.memset(bias2[:], -2.0)

            # startup DMAs: corner-proj operands first, then the rest
            wq16_h = w1.tile([P, KC, HD], F16, tag="wq16h")
            wk16_h = w1.tile([P, KC, HD], F16, tag="wk16h")
            wq8_h = w1.tile([P, KC, HD], F8, tag="wq8h")
            wk8_h = w1.tile([P, KC, HD], F8, tag="wk8h")
            for g in range(4):
                nc.sync.dma_start(wq16_h[:, ts(g, 4), :],
                                  wq16_d[:, 0, ts(g, 4), :])
                nc.sync.dma_start(xT16[:, ts(g, 4), :], xT16_d[:, ts(g, 4), :])
            nc.sync.dma_start(wk16_h[:], wk16_d[:, 0])
            nc.sync.dma_start(wv16[:], wv16_d)
            nc.sync.dma_start(xT8[:], xT8_d)
            nc.sync.dma_start(wq8_h[:], wq8_d[:, 0])
            nc.sync.dma_start(wk8_h[:], wk8_d[:, 0])
            nc.sync.dma_start(wv8[:], wv8_d)
            nc.sync.dma_start(c2[:], c2_d)
            nc.sync.dma_start(s2n[:], s2n_d)
            nc.sync.dma_start(eye16[:], eye_d)
            nc.sync.dma_start(mtri16[:], mtri_d)
            nc.sync.dma_start(mtri2[:], mtri2_d)
            nc.sync.dma_start(ones16[:], ones_d)
            nc.sync.dma_start(oneq[:], oneq_d)
            nc.sync.dma_start(wo16[:], wo16_d)
            nc.sync.dma_start(wo8[:], wo8_d)

            def rope(raw, rot_tag):
                swp = w1.tile([P, S], F16, tag="swap")
                nc.vector.tensor_copy(swp[0:64, :], raw[64:128, :])
                nc.vector.tensor_copy(swp[64:128, :], raw[0:64, :])
                rot = w2.tile([P, S], F16, tag=rot_tag)
                nc.vector.tensor_mul(rot[:], raw[:], c2[:])
                nc.vector.tensor_mul(swp[:], swp[:], s2n[:])
                nc.vector.tensor_add(rot[:], rot[:], swp[:])
                return rot

            def proj_corner(w16h, raw):
                ps = pjp.tile([P, 512], F32, tag="pj")
                for kc in range(KC):
                    nc.tensor.matmul(ps[:], w16h[:, kc, :], xT16[:, kc, :],
                                     start=(kc == 0), stop=(kc == KC - 1))
                nc.scalar.copy(raw[:, 0:512], ps[:])

            def proj_fp8(w8h, raw, c):
                ps = pjp.tile([P, 512], F32, tag="pj")
                for k in range(KC // 2):
                    nc.tensor.matmul(ps[:], w8h[:, 2 * k:2 * k + 2, :],
                                     xT8[:, c - 1, 2 * k:2 * k + 2, :],
                                     start=(k == 0), stop=(k == KC // 2 - 1),
                                     perf_mode=DR)
                nc.vector.tensor_scalar_mul(raw[:, ts(c, 512)], ps[:], 1.0 / WS)

            def v_corner(ti):
                ps = pjp.tile([P, 512], F32, tag="pj")
                for kc in range(KC):
                    nc.tensor.matmul(ps[:], xT16[:, kc, ts(ti, P)],
                                     wv16[:, kc, :],
                                     start=(kc == 0), stop=(kc == KC - 1))
                nc.scalar.copy(V16[:, ti, :], ps[:])
                nc.scalar.copy(V8[:, ti // 2, ti % 2, :], ps[:])

            def v_fp8(ti):
                ps = pjp.tile([P, 512], F32, tag="pj")
                c = ti // 4
                for k in range(KC // 2):
                    nc.tensor.matmul(ps[:], xT8[:, c - 1, 2 * k:2 * k + 2,
                                                 ts(ti % 4, P)],
                                     wv8[:, 2 * k:2 * k + 2, :],
                                     start=(k == 0), stop=(k == KC // 2 - 1),
                                     perf_mode=DR)
                nc.scalar.activation(V8[:, ti // 2, ti % 2, :], ps[:], CPY,
                                     scale=1.0 / WS)

            def load_w(h):
                wq16h = w1.tile([P, KC, HD], F16, tag="wq16h")
                wk16h = w1.tile([P, KC, HD], F16, tag="wk16h")
                wq8h = w1.tile([P, KC, HD], F8, tag="wq8h")
                wk8h = w1.tile([P, KC, HD], F8, tag="wk8h")
                nc.sync.dma_start(wq16h[:], wq16_d[:, h])
                nc.sync.dma_start(wk16h[:], wk16_d[:, h])
                nc.sync.dma_start(wq8h[:], wq8_d[:, h])
                nc.sync.dma_start(wk8h[:], wk8_d[:, h])
                return wq16h, wk16h, wq8h, wk8h

            def full_proj(wq16h, wk16h, wq8h, wk8h, first=False):
                qraw = w1.tile([P, S], F16, tag="qraw")
                kraw = w1.tile([P, S], F16, tag="kraw")
                proj_corner(wq16h, qraw)
                proj_corner(wk16h, kraw)
                if first:
                    for ti in range(4):
                        v_corner(ti)
                for c in range(1, 4):
                    proj_fp8(wq8h, qraw, c)
                    proj_fp8(wk8h, kraw, c)
                    if first:
                        for ti in range(4 * c, 4 * c + 4):
                            v_fp8(ti)
                return rope(qraw, "qrot"), rope(kraw, "krot")

            rots = {0: full_proj(wq16_h, wk16_h, wq8_h, wk8_h, first=True)}

            flip = [False]

            def emit_f16(qi, pool, tail=False):
                for nn in range(4):
                    ps = pool.tile([P, 512], F32, tag="pj")
                    for hh in range(HPC):
                        nc.tensor.matmul(ps[:], OT16[:, hh, ts(qi, P)],
                                         wo16[:, hh, ts(nn, 512)],
                                         start=(hh == 0), stop=(hh == HPC - 1))
                    f_sb = fop.tile([P, 512], F16, tag="fsb")
                    # emit chains are short (~450ns), so an ACT-side copy
                    # only briefly head-of-line-blocks the exp stream;
                    # alternating relieves the DVE queue in the h3 phase
                    if flip[0]:
                        nc.scalar.copy(f_sb[:], ps[:])
                    else:
                        nc.vector.tensor_copy(f_sb[:], ps[:])
                    flip[0] = not flip[0]
                    eng = nc.scalar if tail and nn % 2 else nc.sync
                    eng.dma_start(out_d[:, qi, ts(nn, 512)], f_sb[:])

            def emit_f8(qi, pool, tail=False):
                # qi 1-3 read natural-scale OT8 (qc0 region), rest carry x32
                dsc = 1.0 / WS if qi < 4 else 1.0 / (WS * OS)
                for nn in range(4):
                    ps = pool.tile([P, 512], F32, tag="pj")
                    for j in range(HPC // 2):
                        nc.tensor.matmul(ps[:], OT8[:, 2 * j:2 * j + 2,
                                                     ts(qi, P)],
                                         wo8[:, 2 * j:2 * j + 2, ts(nn, 512)],
                                         start=(j == 0),
                                         stop=(j == HPC // 2 - 1),
                                         perf_mode=DR)
                    f_sb = fop.tile([P, 512], F16, tag="fsb")
                    if flip[0]:
                        nc.scalar.activation(f_sb[:], ps[:], CPY, scale=dsc)
                    else:
                        nc.vector.tensor_scalar_mul(f_sb[:], ps[:], dsc)
                    flip[0] = not flip[0]
                    eng = nc.scalar if tail and nn % 2 else nc.sync
                    eng.dma_start(out_d[:, qi, ts(nn, 512)], f_sb[:])

            # ---- attention ----------------------------------------------
            # Per (h, qc): PV/denominator matmuls run one exp-pair behind the
            # score matmuls so the PE never waits on the ACT exp; next head's
            # projection chains are popped one per pair as PE filler work.
            with tc.tile_pool(name="sp", bufs=2, space="PSUM") as stp, \
                 tc.tile_pool(name="op", bufs=1, space="PSUM") as opp, \
                 tc.tile_pool(name="rp", bufs=1, space="PSUM") as rpp:
                for h in range(HPC):
                    qrot, krot = rots.pop(h)
                    for qc in range(NQ):
                        o_ps = opp.tile([P, 512], F32, tag="o")
                        r_ps = rpp.tile([P, 512], F32, tag="r")
                        qlo, qhi = 512 * qc, 512 * (qc + 1)
                        fq = (list(range(4 * (qc - 1), 4 * qc))
                              if h == HPC - 1 and qc > 0 else [])

                        # pending (kind, kt0, off, e-tile) awaiting PV/denom
                        pend = [None]

                        def flush(last):
                            kind, kt0p, offp, ep = pend[0]
                            if kind == "corner":
                                for i in range(2):
                                    kt = kt0p + i
                                    off = 128 * kt
                                    nc.tensor.matmul(
                                        o_ps[:, off:], V16[:, kt, ts(h, HD)],
                                        ep[:, i, off:],
                                        start=(kt == 0),
                                        stop=(last and i == 1))
                                    nc.tensor.matmul(
                                        r_ps[:, off:], ones16[:],
                                        ep[:, i, off:],
                                        start=(kt == 0),
                                        stop=(last and i == 1))
                            else:
                                nc.tensor.matmul(
                                    o_ps[:, offp:],
                                    V8[:, kt0p // 2, :, ts(h, HD)],
                                    ep[:, :, offp:], start=(kt0p == 0),
                                    stop=last, perf_mode=DR)
                                nc.tensor.matmul(
                                    r_ps[:, offp:], oneq[:], ep[:, :, offp:],
                                    start=(kt0p == 0), stop=last,
                                    perf_mode=DR)
                            pend[0] = None

                        if qc == 0:
                            pairs = [("corner", 2 * j2, 0) for j2 in range(2)]
                        else:
                            pairs = ([("full", 2 * j, 0)
                                      for j in range(2 * qc)] +
                                     [("band", 4 * qc + 2 * j2, 256 * j2)
                                      for j2 in range(2)])
                        for kind, kt0, off in pairs:
                            sp = stp.tile([P, 2, 512], F32, tag="pair")
                            if kind == "corner":
                                for i in range(2):
                                    kt = kt0 + i
                                    o = 128 * kt
                                    nc.tensor.matmul(
                                        sp[:, i, o:], krot[:, ts(kt, P)],
                                        qrot[:, o:512], start=True, stop=False)
                                    nc.tensor.matmul(
                                        sp[:, i, o:o + 128], eye16[:],
                                        mtri16[:], start=False, stop=True)
                                off = 128 * kt0
                            elif kind == "full":
                                for i in range(2):
                                    nc.tensor.matmul(
                                        sp[:, i, :], krot[:, ts(kt0 + i, P)],
                                        qrot[:, qlo:qhi],
                                        start=True, stop=True)
                            else:
                                nc.tensor.matmul(
                                    sp[:, 0, off:], krot[:, ts(kt0, P)],
                                    qrot[:, qlo + off:qhi],
                                    start=True, stop=False)
                                nc.tensor.matmul(
                                    sp[:, 0, off:off + 128], eye16[:],
                                    mtri16[:], start=False, stop=True)
                                nc.tensor.matmul(
                                    sp[:, 1, off:], krot[:, ts(kt0 + 1, P)],
                                    qrot[:, qlo + off:qhi],
                                    start=True, stop=False)
                                nc.tensor.matmul(
                                    sp[:, 1, off:off + 256], eye16[:],
                                    mtri2[:], start=False, stop=True)
                            if kind == "corner":
                                et = etp.tile([P, 2, 512], F16, tag="e16")
                                nc.scalar.activation(et[:, :, off:],
                                                     sp[:, :, off:], EXP,
                                                     scale=SCALE,
                                                     bias=bias4[:])
                            else:
                                et = etp.tile([P, 2, 512], F8, tag="e2")
                                nc.scalar.activation(et[:, :, off:],
                                                     sp[:, :, off:], EXP,
                                                     scale=SCALE,
                                                     bias=bias2[:])
                            if pend[0] is not None:
                                flush(False)
                            pend[0] = (kind, kt0, off, et)
                            if fq:
                                (emit_f16 if fq[0] < 1 else emit_f8)(
                                    fq.pop(0), pjp)
                        flush(True)
                        # evict o_ps to SBUF promptly (ACT) so the PSUM bank
                        # frees without waiting on the rinv->normalize chain;
                        # normalize then runs 2x-mode from SBUF off-path
                        o_sb = fop.tile([P, 512], F16, tag="oraw")
                        nc.scalar.copy(o_sb[:], o_ps[:])
                        rinv = rvp.tile([P, 512], F32, tag="rinv")
                        nc.vector.reciprocal_approx_fast(out=rinv[:],
                                                         in_=r_ps[:])
                        if qc == 0:
                            nc.vector.tensor_mul(OT16[:, h, :], o_sb[:],
                                                 rinv[:])
                            # natural-scale fp8 copy of tokens 128-511 for
                            # the fp8 output projection of qi 1-3
                            nc.vector.tensor_mul(OT8[:, h, 128:512],
                                                 o_sb[:, 128:512],
                                                 rinv[:, 128:512])
                        else:
                            nc.vector.tensor_mul(OT8[:, h, ts(qc, 512)],
                                                 o_sb[:], rinv[:])
                        if h + 1 < HPC and qc == 0:
                            rots[h + 1] = full_proj(*load_w(h + 1))
                    if h == HPC - 1:
                        for qi in range(12, 16):
                            emit_f8(qi, pjp, tail=True)

    nc.compile()
    return nc


def _prep_inputs_causal8(x, wq, wk, wv, wo, freqs_cos, freqs_sin):
    """Build the 8 per-core input maps for the fp8 causal kernel."""
    perm = np.concatenate([np.arange(0, HD, 2), np.arange(1, HD, 2)])

    cosT = freqs_cos.T.astype(np.float32)
    sinT = freqs_sin.T.astype(np.float32)
    c2 = np.concatenate([cosT, cosT], 0).astype(NPF16)
    s2n = np.concatenate([-sinT, sinT], 0).astype(NPF16)

    pp, ff = np.meshgrid(np.arange(P), np.arange(P), indexing="ij")
    mtri = np.where(pp > ff, MASK_NEG, 0.0).astype(NPF16)
    pp2, ff2 = np.meshgrid(np.arange(P), np.arange(256), indexing="ij")
    mtri2 = np.where(ff2 < pp2 + 128, MASK_NEG, 0.0).astype(NPF16)

    common = {
        "c2": c2, "s2n": s2n,
        "eye": np.eye(P, dtype=NPF16),
        "mtri": mtri, "mtri2": mtri2,
        "ones": np.ones((P, P), NPF16),
        "oneq": np.full((P, 2, P), 1.0 / OS, NPF8),
    }

    xT_by_b = []
    for b in range(B):
        xT = np.ascontiguousarray(
            x[b].T.reshape(KC, P, NQ, 512).transpose(1, 2, 0, 3))
        xT_by_b.append((np.ascontiguousarray(xT[:, 0]).astype(NPF16),
                        np.ascontiguousarray(xT[:, 1:]).astype(NPF8)))

    in_maps = []
    for c in range(NCORES):
        b, g = divmod(c, TP)
        heads = range(g * HPC, (g + 1) * HPC)
        cols_qk = np.concatenate([h * HD + perm for h in heads])
        cols_v = np.concatenate([np.arange(h * HD, (h + 1) * HD)
                                 for h in heads])

        wq_c = np.ascontiguousarray(
            wq[:, cols_qk].reshape(KC, P, HPC, HD).transpose(1, 2, 0, 3))
        wk_c = np.ascontiguousarray(
            wk[:, cols_qk].reshape(KC, P, HPC, HD).transpose(1, 2, 0, 3))
        wv_c = np.ascontiguousarray(
            wv[:, cols_v].reshape(KC, P, DVC).transpose(1, 0, 2))
        wo_c = np.ascontiguousarray(
            wo[cols_v, :].reshape(HPC, P, D).transpose(1, 0, 2))

        im = dict(common)
        im["xT16"], im["xT8"] = xT_by_b[b]
        im["wq16"] = wq_c.astype(NPF16)
        im["wk16"] = wk_c.astype(NPF16)
        im["wq8"] = (wq_c * WS).astype(NPF8)
        im["wk8"] = (wk_c * WS).astype(NPF8)
        im["wv16"] = wv_c.astype(NPF16)
        im["wv8"] = (wv_c * WS).astype(NPF8)
        im["wo16"] = wo_c.astype(NPF16)
        im["wo8"] = (wo_c * WS).astype(NPF8)
        in_maps.append(im)
    return in_maps


def _build(mask_mode: str):
    """Build + compile the SPMD program. mask_mode: 'causal'|'none'|'general'."""
    nc = bacc.Bacc("TRN2", target_bir_lowering=False, debug=False,
                   num_devices=NCORES)

    def din(name, shape, dt=F16):
        return nc.dram_tensor(name, shape, dt, kind="ExternalInput").ap()

    xT_d = din("xT", [P, NQ, KC, 512])
    wq_d = din("wq", [P, HPC, KC, HD])
    wk_d = din("wk", [P, HPC, KC, HD])
    wv_d = din("wv", [P, KC, DVC])
    wo_d = din("wo", [P, HPC, D])
    c2_d = din("c2", [P, S])
    s2n_d = din("s2n", [P, S])
    ones_d = din("ones", [P, P])
    if mask_mode == "causal":
        eye_d = din("eye", [P, P])
        mtri_d = din("mtri", [P, P])
    elif mask_mode == "general":
        msk_d = din("expm", [P, NT, S])
    out_d = nc.dram_tensor("out", [P, NT, D], mybir.dt.float16,
                           kind="ExternalOutput").ap()

    with tile.TileContext(nc) as tc:
        with tc.tile_pool(name="static", bufs=1) as st, \
             tc.tile_pool(name="w1", bufs=1) as w1, \
             tc.tile_pool(name="w2", bufs=2) as w2, \
             tc.tile_pool(name="et", bufs=6) as etp, \
             tc.tile_pool(name="ac", bufs=3) as accp, \
             tc.tile_pool(name="fo", bufs=4) as fop, \
             tc.tile_pool(name="pj", bufs=2, space="PSUM") as pjp:

            # ---- static tensors -------------------------------------------
            xT = st.tile([P, NQ, KC, 512], F16, tag="xT")
            wv_sb = st.tile([P, KC, DVC], F16, tag="wv")
            wo_sb = st.tile([P, HPC, D], F16, tag="wo")
            c2 = st.tile([P, S], F16, tag="c2")
            s2n = st.tile([P, S], F16, tag="s2n")
            ones_sb = st.tile([P, P], F16, tag="ones")
            V_sb = st.tile([P, NT, DVC], F16, tag="V")
            OT_sb = st.tile([P, HPC, S], F16, tag="OT")
            if mask_mode == "causal":
                eye_sb = st.tile([P, P], F16, tag="eye")
                mtri_sb = st.tile([P, P], F16, tag="mtri")
            bias4 = st.tile([P, 1], F32, tag="b4")
            nc.vector.memset(bias4[:], -4.0)

            # head-0 weights first (small), then interleaved wv/xT chunks so
            # the V-phase matmuls can start as soon as chunk 0 lands.
            wq_h = w1.tile([P, KC, HD], F16, tag="wqh")
            wk_h = w1.tile([P, KC, HD], F16, tag="wkh")
            for g in range(4):
                nc.sync.dma_start(wq_h[:, ts(g, 4), :], wq_d[:, 0, ts(g, 4), :])
                nc.sync.dma_start(xT[:, 0, ts(g, 4), :], xT_d[:, 0, ts(g, 4), :])
            nc.sync.dma_start(wk_h[:], wk_d[:, 0])
            nc.sync.dma_start(wv_sb[:], wv_d)
            nc.sync.dma_start(xT[:, 1, :, :], xT_d[:, 1, :, :])
            nc.sync.dma_start(xT[:, 2, :, :], xT_d[:, 2, :, :])
            nc.sync.dma_start(xT[:, 3, :, :], xT_d[:, 3, :, :])
            nc.sync.dma_start(c2[:], c2_d)
            nc.sync.dma_start(s2n[:], s2n_d)
            nc.sync.dma_start(ones_sb[:], ones_d)
            if mask_mode == "causal":
                nc.sync.dma_start(eye_sb[:], eye_d)
                nc.sync.dma_start(mtri_sb[:], mtri_d)
            nc.sync.dma_start(wo_sb[:], wo_d)

            def proj_half(w_h, raw_tag, rot_tag):
                """One projection (Q or K) + RoPE -> rotated [HD, S] tile."""
                raw = w1.tile([P, S], F16, tag=raw_tag)
                for t in range(NQ):
                    ps = pjp.tile([P, 512], F32, tag="pj")
                    for kc in range(KC):
                        nc.tensor.matmul(ps[:], w_h[:, kc, :],
                                         xT[:, t, kc, :],
                                         start=(kc == 0), stop=(kc == KC - 1))
                    nc.scalar.copy(raw[:, ts(t, 512)], ps[:])
                return rope(raw, rot_tag)

            def load_w(h):
                wq_h = w1.tile([P, KC, HD], F16, tag="wqh")
                nc.sync.dma_start(wq_h[:], wq_d[:, h])
                wk_h = w1.tile([P, KC, HD], F16, tag="wkh")
                nc.sync.dma_start(wk_h[:], wk_d[:, h])
                return wq_h, wk_h

            # head-0 projections first, Q/K interleaved per token chunk so the
            # PE consumes xT chunks as the startup DMAs land; the RoPE chains
            # then run during the V phase.
            def rope(raw, rot_tag):
                swp = w1.tile([P, S], F16, tag="swap")
                nc.vector.tensor_copy(swp[0:64, :], raw[64:128, :])
                nc.vector.tensor_copy(swp[64:128, :], raw[0:64, :])
                rot = w2.tile([P, S], F16, tag=rot_tag)
                nc.vector.tensor_mul(rot[:], raw[:], c2[:])
                nc.vector.tensor_mul(swp[:], swp[:], s2n[:])
                nc.vector.tensor_add(rot[:], rot[:], swp[:])
                return rot

            qraw0 = w1.tile([P, S], F16, tag="qraw")
            kraw0 = w1.tile([P, S], F16, tag="kraw")
            for t in range(NQ):
                for w_h, raw in ((wq_h, qraw0), (wk_h, kraw0)):
                    ps = pjp.tile([P, 512], F32, tag="pj")
                    for kc in range(KC):
                        nc.tensor.matmul(ps[:], w_h[:, kc, :], xT[:, t, kc, :],
                                         start=(kc == 0), stop=(kc == KC - 1))
                    nc.scalar.copy(raw[:, ts(t, 512)], ps[:])
                # V projection for this token chunk keeps the PE busy while
                # the next xT chunk is still streaming in
                for ti in range(4 * t, 4 * t + 4):
                    ps = pjp.tile([P, 512], F32, tag="pj")
                    for kc in range(KC):
                        nc.tensor.matmul(ps[:], xT[:, t, kc, ts(ti % 4, P)],
                                         wv_sb[:, kc, :],
                                         start=(kc == 0), stop=(kc == KC - 1))
                    nc.scalar.copy(V_sb[:, ti, :], ps[:])
            rots = {0: (rope(qraw0, "qrot"), rope(kraw0, "krot"))}

            flip = [False]

            def emit_f(qi, pool, scalar_only=False):
                for nn in range(D // 512):
                    ps = pool.tile([P, 512], F32, tag="pj")
                    for hh in range(HPC):
                        nc.tensor.matmul(ps[:], OT_sb[:, hh, ts(qi, P)],
                                         wo_sb[:, hh, ts(nn, 512)],
                                         start=(hh == 0), stop=(hh == HPC - 1))
                    f_sb = fop.tile([P, 512], F16, tag="fsb")
                    # keep the copies off the Vector engine when F is inlined
                    # into attention: they would head-of-line-block the acc
                    # adds in DVE's in-order queue
                    if flip[0] and not scalar_only:
                        nc.vector.tensor_copy(f_sb[:], ps[:])
                    else:
                        nc.scalar.copy(f_sb[:], ps[:])
                    flip[0] = not flip[0]
                    nc.sync.dma_start(out_d[:, qi, ts(nn, 512)], f_sb[:])

            # ---- attention, with next head's projections interleaved ------
            with tc.tile_pool(name="sp", bufs=3, space="PSUM") as stp, \
                 tc.tile_pool(name="op", bufs=2, space="PSUM") as opp, \
                 tc.tile_pool(name="rp", bufs=1, space="PSUM") as rpp:
                for h in range(HPC):
                    qrot, krot = rots.pop(h)
                    for qc in range(NQ):
                        o_ps = opp.tile([P, 512], F32, tag="o")
                        r_ps = rpp.tile([P, 512], F32, tag="r")
                        nkt = 4 * (qc + 1) if mask_mode == "causal" else NT
                        nfull = 4 * qc if mask_mode == "causal" else 0
                        # previous chunk's output-projection tiles, spread one
                        # per kt iteration so their PSUM->SBUF copies pace
                        # evenly through both engines' queues
                        fq = (list(range(4 * (qc - 1), 4 * qc))
                              if h == HPC - 1 and mask_mode == "causal" and qc > 0
                              else [])
                        # full-width tiles accumulate on the DVE into `acc`;
                        # one ones-matmul on the sum replaces one per tile.
                        acc = first_e = None
                        for kt in range(nkt):
                            band = mask_mode == "causal" and kt >= nfull
                            off = 128 * (kt - nfull) if band else 0
                            s_ps = stp.tile([P, 512], F32, tag="s")
                            nc.tensor.matmul(
                                s_ps[:, off:], krot[:, ts(kt, P)],
                                qrot[:, 512 * qc + off: 512 * (qc + 1)],
                                start=True, stop=not band)
                            if band:
                                nc.tensor.matmul(
                                    s_ps[:, off:off + 128], eye_sb[:], mtri_sb[:],
                                    start=False, stop=True)
                            eT = etp.tile([P, 512], F16, tag="e")
                            # bias -4 (softmax is shift-invariant; the ones-
                            # matmul denominator absorbs it) keeps exp outputs
                            # well inside fp16 range even for hot scores
                            nc.scalar.activation(eT[:, off:], s_ps[:, off:], EXP,
                                                 scale=SCALE, bias=bias4[:])
                            if mask_mode == "general":
                                em = etp.tile([P, 512], F16, tag="em")
                                nc.sync.dma_start(em[:], msk_d[:, kt, ts(qc, 512)])
                                nc.gpsimd.tensor_mul(eT[:], eT[:], em[:])
                            nc.tensor.matmul(o_ps[:, off:],
                                             V_sb[:, kt, ts(h, HD)], eT[:, off:],
                                             start=(kt == 0), stop=(kt == nkt - 1))
                            if mask_mode != "causal":
                                nc.tensor.matmul(r_ps[:], ones_sb[:], eT[:],
                                                 start=(kt == 0),
                                                 stop=(kt == nkt - 1))
                            elif not band:
                                if first_e is not None:
                                    acc = accp.tile([P, 512], F16, tag="acc")
                                    nc.vector.tensor_add(acc[:], first_e[:], eT[:])
                                    first_e = None
                                elif acc is not None:
                                    nc.vector.tensor_add(acc[:], acc[:], eT[:])
                                else:
                                    first_e = eT
                            elif acc is None and first_e is None:
                                acc = accp.tile([P, 512], F16, tag="acc")
                                nc.vector.tensor_copy(acc[:], eT[:])
                            elif first_e is not None:
                                acc = accp.tile([P, 512], F16, tag="acc")
                                nc.vector.tensor_add(acc[:], first_e[:], eT[:])
                                first_e = None
                            else:
                                nc.vector.tensor_add(acc[:, off:], acc[:, off:],
                                                     eT[:, off:])
                            if fq and kt % 3 == 2:
                                emit_f(fq.pop(0), pjp)
                        # leftover output-projection tiles go before the
                        # denominator matmul: they keep the PE busy while the
                        # Scalar engine finishes the trailing band exps
                        for qi in fq:
                            emit_f(qi, pjp)
                        if acc is not None:
                            nc.tensor.matmul(r_ps[:], ones_sb[:], acc[:],
                                             start=True, stop=True)
                        rinv = fop.tile([P, 512], F32, tag="rinv")
                        nc.vector.reciprocal_approx_fast(out=rinv[:], in_=r_ps[:])
                        nc.vector.tensor_mul(OT_sb[:, h, ts(qc, 512)], o_ps[:],
                                             rinv[:])
                        # pipeline the next head's projections + RoPE
                        if h + 1 < HPC and qc == 0:
                            wq_n, wk_n = load_w(h + 1)
                            rots[h + 1] = (proj_half(wq_n, "qraw", "qrot"),
                                           proj_half(wk_n, "kraw", "krot"))
                    if h == HPC - 1 and mask_mode == "causal":
                        for qi in range(4 * (NQ - 1), NT):
                            emit_f(qi, pjp)

            # ---- output projection for non-causal modes (causal streams it
            # inside the last head's attention) --------------------------------
            if mask_mode != "causal":
                with tc.tile_pool(name="fp", bufs=6, space="PSUM") as fpp:
                    for qi in range(NT):
                        emit_f(qi, fpp)

    nc.compile()
    return nc


def _get_program(mask_mode: str):
    if mask_mode not in _cache:
        if mask_mode == "causal":
            _cache[mask_mode] = _build_causal8()
        else:
            _cache[mask_mode] = _build(mask_mode)
    return _cache[mask_mode]


def _detect_mask_mode(mask: np.ndarray) -> str:
    m = mask.reshape(S, S)
    iu = np.triu_indices(S, 1)
    upper = m[iu]
    lower_ok = np.max(np.abs(np.tril(m))) == 0.0
    if lower_ok and upper.size and np.all(upper <= -1e8):
        return "causal"
    if np.max(np.abs(m)) == 0.0:
        return "none"
    return "general"


def _prep_inputs(x, wq, wk, wv, wo, freqs_cos, freqs_sin, mask, mask_mode):
    """Build the 8 per-core input maps (host-side sharding + layout)."""
    # within-head even/odd permutation so RoPE pairs land in partition halves
    perm = np.concatenate([np.arange(0, HD, 2), np.arange(1, HD, 2)])

    cosT = freqs_cos.T.astype(np.float32)          # [64, S]
    sinT = freqs_sin.T.astype(np.float32)
    c2 = np.concatenate([cosT, cosT], 0).astype(NPF16)     # [128, S]
    s2n = np.concatenate([-sinT, sinT], 0).astype(NPF16)
    ones = np.ones((P, P), NPF16)

    common = {"c2": c2, "s2n": s2n, "ones": ones}
    if mask_mode == "causal":
        common["eye"] = np.eye(P, dtype=NPF16)
        pp, ff = np.meshgrid(np.arange(P), np.arange(P), indexing="ij")
        common["mtri"] = np.where(pp > ff, MASK_NEG, 0.0).astype(NPF16)
    elif mask_mode == "general":
        m = mask.reshape(S, S).astype(np.float32)
        # eT[kt_tok, qt_tok] is multiplied by exp(SCALE * mask[qt_tok, kt_tok])
        expm = np.exp(SCALE * m.T).astype(NPF16)            # [k_tok, q_tok]
        common["expm"] = np.ascontiguousarray(
            expm.reshape(NT, P, S).transpose(1, 0, 2))

    xT_by_b = []
    for b in range(B):
        xT = np.ascontiguousarray(
            x[b].T.reshape(KC, P, NQ, 512).transpose(1, 2, 0, 3)).astype(NPF16)
        xT_by_b.append(xT)

    in_maps = []
    for c in range(NCORES):
        b, g = divmod(c, TP)
        heads = range(g * HPC, (g + 1) * HPC)
        cols_qk = np.concatenate([h * HD + perm for h in heads])
        cols_v = np.concatenate([np.arange(h * HD, (h + 1) * HD) for h in heads])

        wq_c = wq[:, cols_qk].reshape(KC, P, HPC, HD).transpose(1, 2, 0, 3)
        wk_c = wk[:, cols_qk].reshape(KC, P, HPC, HD).transpose(1, 2, 0, 3)
        wv_c = wv[:, cols_v].reshape(KC, P, DVC).transpose(1, 0, 2)
        wo_c = wo[cols_v, :].reshape(HPC, P, D).transpose(1, 0, 2)

        im = dict(common)
        im["xT"] = xT_by_b[b]
        im["wq"] = np.ascontiguousarray(wq_c).astype(NPF16)
        im["wk"] = np.ascontiguousarray(wk_c).astype(NPF16)
        im["wv"] = np.ascontiguousarray(wv_c).astype(NPF16)
        im["wo"] = np.ascontiguousarray(wo_c).astype(NPF16)
        in_maps.append(im)
    return in_maps


def run(inputs: dict, **spmd_kwargs):
    """Run on hardware; returns (output [B,S,D] fp32, BassKernelResults)."""
    x = np.asarray(inputs["x"], np.float32)
    wq = np.asarray(inputs["wq"], np.float32)
    wk = np.asarray(inputs["wk"], np.float32)
    wv = np.asarray(inputs["wv"], np.float32)
    wo = np.asarray(inputs["wo"], np.float32)
    fc = np.asarray(inputs["freqs_cos"], np.float32)
    fs = np.asarray(inputs["freqs_sin"], np.float32)
    mask = np.asarray(inputs["mask"], np.float32)

    mask_mode = _detect_mask_mode(mask)
    nc = _get_program(mask_mode)
    if mask_mode == "causal":
        in_maps = _prep_inputs_causal8(x, wq, wk, wv, wo, fc, fs)
    else:
        in_maps = _prep_inputs(x, wq, wk, wv, wo, fc, fs, mask, mask_mode)
    res = run_bass_kernel_spmd(nc, in_maps, core_ids=list(range(NCORES)),
                               **spmd_kwargs)

    out = np.zeros((B, S, D), np.float32)
    for c in range(NCORES):
        b = c // TP
        part = res.results[c]["out"].astype(np.float32)   # [P, NT, D]
        out[b] += part.transpose(1, 0, 2).reshape(S, D)
    return out, res


def kernel(**inputs) -> np.ndarray:
    out, _ = run(inputs)
    return out



# revision 29
# speedup vs baseline: 1.1786x; 1.0053x over previous
"""Trainium2 Bass kernel for nn_Attention_62620623176132.

Multi-head causal attention with RoPE (LLaMA-style), B=2, S=2048, D=2048,
H=16 heads of HD=128, fp32 reference.

Sharding (hardcoded): 8 cores = 2-way data parallel over batch x 4-way
tensor parallel over heads (4 heads per core). Each core computes its 4
heads' Q/K/V projections, attention, and a partial output projection
(rows of wo for its heads); the host sums the 4 fp16 partials per batch
in fp32.

Causal path (_build_causal8): mixed fp16/fp8e4 precision. The causal
output's absmax sits at ~41 sigma on the earliest tokens (they average
few values), so query-chunk 0 (tokens 0-511) is computed fully in fp16;
queries 512+ use fp8e4 DoubleRow matmuls wherever the contraction can be
doubled (QKV projections, PV + denominator key-tile pairs, output
projection). Scores stay fp16 (contraction HD=128; DoubleRow measured
1.0x there). fp8 weights are pre-scaled x64; descales fold into the
PSUM evictions. Measured ~250us vs 327us for the all-fp16 kernel, rel
err 7e-3 vs the 2e-2 gate.

Legacy fp16 algorithm (kept for the non-causal fallback paths):
  - x^T kept SBUF-resident; Q^T/K^T computed per head in [HD, S] layout,
    V in [S, dv] layout, so no transposes are ever needed.
  - RoPE via host-side even/odd column permutation of wq/wk: rotation
    pairs land in partition halves; 3 DVE tensor ops + 2 swap copies.
  - Scores computed transposed, sT[kt, qt] = kT . qT, so exp(sT) feeds
    the PV matmul directly as the moving operand. exp is shifted by -4
    (softmax is shift-invariant) to keep fp16 outputs far from overflow.
  - Softmax denominators: exp tiles are summed on the Vector engine and
    one all-ones stationary matmul per query chunk broadcasts the column
    sums to all partitions; the normalization multiply is fused into the
    PSUM->SBUF copy of the attention output.
  - Causality: score tiles above the diagonal are skipped; band tiles
    are restricted to their unmasked columns and the diagonal square
    gets -60000 added in PSUM by one extra matmul (identity x triangle),
    so masking costs no vector-engine work.
  - Projections for head h+1 are emitted inside head h's attention so
    the serial RoPE chain never stalls the PE; the output projection is
    streamed inside the last head's attention the same way.
  - Startup DMAs are ordered/split so the first projection matmuls start
    as soon as the first x^T chunk lands.

Fallback paths keyed off the runtime mask: all-zero mask -> non-causal
kernel; any other mask -> multiplicative exp(mask/sqrt(HD)) tiles
streamed from DRAM (correct for arbitrary masks, slower).
"""

import math

import numpy as np
import ml_dtypes
import concourse.tile as tile
import concourse.mybir as mybir
from concourse import bacc
from concourse.bass import ts
from concourse.bass_utils import run_bass_kernel_spmd

B, S, D, H, HD = 2, 2048, 2048, 16, 128
P = 128
NCORES = 8
TP = 4                # head-parallel groups
HPC = H // TP         # heads per core = 4
DVC = HPC * HD        # 512 v-dims per core
KC = D // P           # 16 contraction chunks
NT = S // P           # 16 token tiles of 128
NQ = S // 512         # 4 query chunks of 512
F8 = mybir.dt.float8e4
F16 = mybir.dt.float16
F32 = mybir.dt.float32
NPF8 = ml_dtypes.float8_e4m3
NPF16 = np.float16
MASK_NEG = -60000.0
SCALE = 1.0 / math.sqrt(HD)
WS = 64.0             # fp8 weight pre-scale (power of 2, exact)
OS = 32.0             # OT8 scale, via oneq = 1/32 denominator operand
EXP = mybir.ActivationFunctionType.Exp
CPY = mybir.ActivationFunctionType.Copy
DR = mybir.MatmulPerfMode.DoubleRow

_cache: dict = {}


def _build_causal8():
    """fp8 causal kernel: query-chunk 0 (tokens 0-511) fully fp16 (outputs
    there reach ~40 sigma and need low relative error); queries 512+ use
    fp8e4 DoubleRow matmuls wherever the contraction doubles (projections,
    PV pairs, denominator pairs, output projection). Scores stay fp16:
    their contraction is HD=128 and DoubleRow cannot help (measured 1.0x).

    fp8 weights are pre-scaled x64 (exact); 1/64 descale is folded into
    PSUM->SBUF copies. OT8 carries x32 via the 1/32 'oneq' denominator
    matmul; the x2048 on fp8 output-projection PSUM descales in the final
    copy."""
    nc = bacc.Bacc("TRN2", target_bir_lowering=False, debug=False,
                   num_devices=NCORES)

    def din(name, shape, dt):
        return nc.dram_tensor(name, shape, dt, kind="ExternalInput").ap()

    xT16_d = din("xT16", [P, KC, 512], F16)
    xT8_d = din("xT8", [P, 3, KC, 512], F8)
    wq16_d = din("wq16", [P, HPC, KC, HD], F16)
    wk16_d = din("wk16", [P, HPC, KC, HD], F16)
    wq8_d = din("wq8", [P, HPC, KC, HD], F8)
    wk8_d = din("wk8", [P, HPC, KC, HD], F8)
    wv16_d = din("wv16", [P, KC, DVC], F16)
    wv8_d = din("wv8", [P, KC, DVC], F8)
    wo16_d = din("wo16", [P, HPC, D], F16)
    wo8_d = din("wo8", [P, HPC, D], F8)
    c2_d = din("c2", [P, S], F16)
    s2n_d = din("s2n", [P, S], F16)
    eye_d = din("eye", [P, P], F16)
    mtri_d = din("mtri", [P, P], F16)
    mtri2_d = din("mtri2", [P, 256], F16)
    ones_d = din("ones", [P, P], F16)
    oneq_d = din("oneq", [P, 2, P], F8)
    out_d = nc.dram_tensor("out", [P, NT, D], F16, kind="ExternalOutput").ap()

    with tile.TileContext(nc) as tc:
        with tc.tile_pool(name="static", bufs=1) as st, \
             tc.tile_pool(name="w1", bufs=1) as w1, \
             tc.tile_pool(name="w2", bufs=2) as w2, \
             tc.tile_pool(name="et", bufs=4) as etp, \
             tc.tile_pool(name="fo", bufs=4) as fop, \
             tc.tile_pool(name="rv", bufs=2) as rvp, \
             tc.tile_pool(name="pj", bufs=2, space="PSUM") as pjp:

            # ---- static tensors ------------------------------------------
            xT16 = st.tile([P, KC, 512], F16, tag="xT16")
            xT8 = st.tile([P, 3, KC, 512], F8, tag="xT8")
            wv16 = st.tile([P, KC, DVC], F16, tag="wv16")
            wv8 = st.tile([P, KC, DVC], F8, tag="wv8")
            wo16 = st.tile([P, HPC, D], F16, tag="wo16")
            wo8 = st.tile([P, HPC, D], F8, tag="wo8")
            c2 = st.tile([P, S], F16, tag="c2")
            s2n = st.tile([P, S], F16, tag="s2n")
            eye16 = st.tile([P, P], F16, tag="eye")
            mtri16 = st.tile([P, P], F16, tag="mtri")
            mtri2 = st.tile([P, 256], F16, tag="mtri2")
            ones16 = st.tile([P, P], F16, tag="ones")
            oneq = st.tile([P, 2, P], F8, tag="oneq")
            V16 = st.tile([P, 4, DVC], F16, tag="V16")
            V8 = st.tile([P, NT // 2, 2, DVC], F8, tag="V8")
            OT16 = st.tile([P, HPC, 512], F16, tag="OT16")
            OT8 = st.tile([P, HPC, S], F8, tag="OT8")
            bias4 = st.tile([P, 1], F32, tag="b4")
            bias2 = st.tile([P, 1], F32, tag="b2")
            nc.vector.memset(bias4[:], -4.0)
            nc.vector.memset(bias2[:], -2.0)

            # startup DMAs: corner-proj operands first, then the rest
            wq16_h = w1.tile([P, KC, HD], F16, tag="wq16h")
            wk16_h = w1.tile([P, KC, HD], F16, tag="wk16h")
            wq8_h = w1.tile([P, KC, HD], F8, tag="wq8h")
            wk8_h = w1.tile([P, KC, HD], F8, tag="wk8h")
            for g in range(4):
                nc.sync.dma_start(wq16_h[:, ts(g, 4), :],
                                  wq16_d[:, 0, ts(g, 4), :])
                nc.sync.dma_start(xT16[:, ts(g, 4), :], xT16_d[:, ts(g, 4), :])
            nc.sync.dma_start(wk16_h[:], wk16_d[:, 0])
            nc.sync.dma_start(wv16[:], wv16_d)
            nc.sync.dma_start(xT8[:], xT8_d)
            nc.sync.dma_start(wq8_h[:], wq8_d[:, 0])
            nc.sync.dma_start(wk8_h[:], wk8_d[:, 0])
            nc.sync.dma_start(wv8[:], wv8_d)
            nc.sync.dma_start(c2[:], c2_d)
            nc.sync.dma_start(s2n[:], s2n_d)
            nc.sync.dma_start(eye16[:], eye_d)
            nc.sync.dma_start(mtri16[:], mtri_d)
            nc.sync.dma_start(mtri2[:], mtri2_d)
            nc.sync.dma_start(ones16[:], ones_d)
            nc.sync.dma_start(oneq[:], oneq_d)
            nc.sync.dma_start(wo16[:], wo16_d)
            nc.sync.dma_start(wo8[:], wo8_d)

            def rope(raw, rot_tag):
                swp = w1.tile([P, S], F16, tag="swap")
                nc.vector.tensor_copy(swp[0:64, :], raw[64:128, :])
                nc.vector.tensor_copy(swp[64:128, :], raw[0:64, :])
                rot = w2.tile([P, S], F16, tag=rot_tag)
                nc.vector.tensor_mul(rot[:], raw[:], c2[:])
                nc.vector.tensor_mul(swp[:], swp[:], s2n[:])
                nc.vector.tensor_add(rot[:], rot[:], swp[:])
                return rot

            def proj_corner(w16h, raw):
                ps = pjp.tile([P, 512], F32, tag="pj")
                for kc in range(KC):
                    nc.tensor.matmul(ps[:], w16h[:, kc, :], xT16[:, kc, :],
                                     start=(kc == 0), stop=(kc == KC - 1))
                nc.scalar.copy(raw[:, 0:512], ps[:])

            def proj_fp8_mm(w8h, c):
                ps = pjp.tile([P, 512], F32, tag="pj")
                for k in range(KC // 2):
                    nc.tensor.matmul(ps[:], w8h[:, 2 * k:2 * k + 2, :],
                                     xT8[:, c - 1, 2 * k:2 * k + 2, :],
                                     start=(k == 0), stop=(k == KC // 2 - 1),
                                     perf_mode=DR)
                return ps

            def proj_fp8(w8h, raw, c):
                ps = proj_fp8_mm(w8h, c)
                nc.vector.tensor_scalar_mul(raw[:, ts(c, 512)], ps[:], 1.0 / WS)

            def v_corner(ti):
                ps = pjp.tile([P, 512], F32, tag="pj")
                for kc in range(KC):
                    nc.tensor.matmul(ps[:], xT16[:, kc, ts(ti, P)],
                                     wv16[:, kc, :],
                                     start=(kc == 0), stop=(kc == KC - 1))
                nc.scalar.copy(V16[:, ti, :], ps[:])
                nc.scalar.copy(V8[:, ti // 2, ti % 2, :], ps[:])

            def v_fp8(ti):
                ps = pjp.tile([P, 512], F32, tag="pj")
                c = ti // 4
                for k in range(KC // 2):
                    nc.tensor.matmul(ps[:], xT8[:, c - 1, 2 * k:2 * k + 2,
                                                 ts(ti % 4, P)],
                                     wv8[:, 2 * k:2 * k + 2, :],
                                     start=(k == 0), stop=(k == KC // 2 - 1),
                                     perf_mode=DR)
                nc.scalar.activation(V8[:, ti // 2, ti % 2, :], ps[:], CPY,
                                     scale=1.0 / WS)

            def load_w(h):
                wq16h = w1.tile([P, KC, HD], F16, tag="wq16h")
                wk16h = w1.tile([P, KC, HD], F16, tag="wk16h")
                wq8h = w1.tile([P, KC, HD], F8, tag="wq8h")
                wk8h = w1.tile([P, KC, HD], F8, tag="wk8h")
                nc.sync.dma_start(wq16h[:], wq16_d[:, h])
                nc.sync.dma_start(wk16h[:], wk16_d[:, h])
                nc.sync.dma_start(wq8h[:], wq8_d[:, h])
                nc.sync.dma_start(wk8h[:], wk8_d[:, h])
                return wq16h, wk16h, wq8h, wk8h

            def full_proj(wq16h, wk16h, wq8h, wk8h, first=False):
                qraw = w1.tile([P, S], F16, tag="qraw")
                kraw = w1.tile([P, S], F16, tag="kraw")
                proj_corner(wq16h, qraw)
                proj_corner(wk16h, kraw)
                if first:
                    for ti in range(4):
                        v_corner(ti)
                for c in range(1, 4):
                    proj_fp8(wq8h, qraw, c)
                    proj_fp8(wk8h, kraw, c)
                    if first:
                        for ti in range(4 * c, 4 * c + 4):
                            v_fp8(ti)
                return rope(qraw, "qrot"), rope(kraw, "krot")

            rots = {0: full_proj(wq16_h, wk16_h, wq8_h, wk8_h, first=True)}

            flip = [False]

            def emit_f16(qi, pool, tail=False):
                for nn in range(4):
                    ps = pool.tile([P, 512], F32, tag="pj")
                    for hh in range(HPC):
                        nc.tensor.matmul(ps[:], OT16[:, hh, ts(qi, P)],
                                         wo16[:, hh, ts(nn, 512)],
                                         start=(hh == 0), stop=(hh == HPC - 1))
                    f_sb = fop.tile([P, 512], F16, tag="fsb")
                    # emit chains are short (~450ns), so an ACT-side copy
                    # only briefly head-of-line-blocks the exp stream;
                    # alternating relieves the DVE queue in the h3 phase
                    if flip[0]:
                        nc.scalar.copy(f_sb[:], ps[:])
                    else:
                        nc.vector.tensor_copy(f_sb[:], ps[:])
                    flip[0] = not flip[0]
                    eng = nc.scalar if tail and nn % 2 else nc.sync
                    eng.dma_start(out_d[:, qi, ts(nn, 512)], f_sb[:])

            def emit_f8(qi, pool, tail=False):
                # qi 1-3 read natural-scale OT8 (qc0 region), rest carry x32
                dsc = 1.0 / WS if qi < 4 else 1.0 / (WS * OS)
                for nn in range(4):
                    ps = pool.tile([P, 512], F32, tag="pj")
                    for j in range(HPC // 2):
                        nc.tensor.matmul(ps[:], OT8[:, 2 * j:2 * j + 2,
                                                     ts(qi, P)],
                                         wo8[:, 2 * j:2 * j + 2, ts(nn, 512)],
                                         start=(j == 0),
                                         stop=(j == HPC // 2 - 1),
                                         perf_mode=DR)
                    f_sb = fop.tile([P, 512], F16, tag="fsb")
                    if flip[0]:
                        nc.scalar.activation(f_sb[:], ps[:], CPY, scale=dsc)
                    else:
                        nc.vector.tensor_scalar_mul(f_sb[:], ps[:], dsc)
                    flip[0] = not flip[0]
                    eng = nc.scalar if tail and nn % 2 else nc.sync
                    eng.dma_start(out_d[:, qi, ts(nn, 512)], f_sb[:])

            # ---- attention ----------------------------------------------
            # Per (h, qc): PV/denominator matmuls run one exp-pair behind the
            # score matmuls so the PE never waits on the ACT exp; next head's
            # projection chains are popped one per pair as PE filler work.
            with tc.tile_pool(name="sp", bufs=2, space="PSUM") as stp, \
                 tc.tile_pool(name="op", bufs=1, space="PSUM") as opp, \
                 tc.tile_pool(name="rp", bufs=1, space="PSUM") as rpp:
                for h in range(HPC):
                    qrot, krot = rots.pop(h)
                    # fp8 proj chains for h+1 (chunks 1-2), interleaved into
                    # qc1/qc2 pairs as PE filler; their PSUM evictions are
                    # DEFERRED to the qc boundary so no chain-dependent copy
                    # ever sits between exps in the ACT/DVE FIFOs
                    dchains = []
                    held = []
                    ropef = [None]
                    for qc in range(NQ):
                        o_ps = opp.tile([P, 512], F32, tag="o")
                        r_ps = rpp.tile([P, 512], F32, tag="r")
                        qlo, qhi = 512 * qc, 512 * (qc + 1)
                        fq = (list(range(4 * (qc - 1), 4 * qc))
                              if h == HPC - 1 and qc > 0 else [])

                        # pending (kind, kt0, off, e-tile) awaiting PV/denom
                        pend = [None]

                        def flush(last):
                            kind, kt0p, offp, ep = pend[0]
                            if kind == "corner":
                                for i in range(2):
                                    kt = kt0p + i
                                    off = 128 * kt
                                    nc.tensor.matmul(
                                        o_ps[:, off:], V16[:, kt, ts(h, HD)],
                                        ep[:, i, off:],
                                        start=(kt == 0),
                                        stop=(last and i == 1))
                                    nc.tensor.matmul(
                                        r_ps[:, off:], ones16[:],
                                        ep[:, i, off:],
                                        start=(kt == 0),
                                        stop=(last and i == 1))
                            else:
                                nc.tensor.matmul(
                                    o_ps[:, offp:],
                                    V8[:, kt0p // 2, :, ts(h, HD)],
                                    ep[:, :, offp:], start=(kt0p == 0),
                                    stop=last, perf_mode=DR)
                                nc.tensor.matmul(
                                    r_ps[:, offp:], oneq[:], ep[:, :, offp:],
                                    start=(kt0p == 0), stop=last,
                                    perf_mode=DR)
                            pend[0] = None

                        if qc == 0:
                            pairs = [("corner", 2 * j2, 0) for j2 in range(2)]
                        else:
                            pairs = ([("full", 2 * j, 0)
                                      for j in range(2 * qc)] +
                                     [("band", 4 * qc + 2 * j2, 256 * j2)
                                      for j2 in range(2)])
                        for pi, (kind, kt0, off) in enumerate(pairs):
                            sp = stp.tile([P, 2, 512], F32, tag="pair")
                            if kind == "corner":
                                for i in range(2):
                                    kt = kt0 + i
                                    o = 128 * kt
                                    nc.tensor.matmul(
                                        sp[:, i, o:], krot[:, ts(kt, P)],
                                        qrot[:, o:512], start=True, stop=False)
                                    nc.tensor.matmul(
                                        sp[:, i, o:o + 128], eye16[:],
                                        mtri16[:], start=False, stop=True)
                                off = 128 * kt0
                            elif kind == "full":
                                for i in range(2):
                                    nc.tensor.matmul(
                                        sp[:, i, :], krot[:, ts(kt0 + i, P)],
                                        qrot[:, qlo:qhi],
                                        start=True, stop=True)
                            else:
                                nc.tensor.matmul(
                                    sp[:, 0, off:], krot[:, ts(kt0, P)],
                                    qrot[:, qlo + off:qhi],
                                    start=True, stop=False)
                                nc.tensor.matmul(
                                    sp[:, 0, off:off + 128], eye16[:],
                                    mtri16[:], start=False, stop=True)
                                nc.tensor.matmul(
                                    sp[:, 1, off:], krot[:, ts(kt0 + 1, P)],
                                    qrot[:, qlo + off:qhi],
                                    start=True, stop=False)
                                nc.tensor.matmul(
                                    sp[:, 1, off:off + 256], eye16[:],
                                    mtri2[:], start=False, stop=True)
                            if kind == "corner":
                                et = etp.tile([P, 2, 512], F16, tag="e16")
                                nc.scalar.activation(et[:, :, off:],
                                                     sp[:, :, off:], EXP,
                                                     scale=SCALE,
                                                     bias=bias4[:])
                            else:
                                et = etp.tile([P, 2, 512], F8, tag="e2")
                                nc.scalar.activation(et[:, :, off:],
                                                     sp[:, :, off:], EXP,
                                                     scale=SCALE,
                                                     bias=bias2[:])
                            if pend[0] is not None:
                                flush(False)
                            pend[0] = (kind, kt0, off, et)
                            if fq:
                                (emit_f16 if fq[0] < 1 else emit_f8)(
                                    fq.pop(0), pjp)
                            elif dchains and 1 <= pi <= 2 and len(held) < 2:
                                w8h, raw_t, c = dchains.pop(0)
                                held.append((proj_fp8_mm(w8h, c), raw_t, c))
                        # deferred chain evictions at the qc boundary
                        for ps_h, raw_t, c in held:
                            nc.vector.tensor_scalar_mul(raw_t[:, ts(c, 512)],
                                                        ps_h[:], 1.0 / WS)
                        held.clear()
                        if qc == 2 and ropef[0] is not None:
                            ropef[0]()
                            ropef[0] = None
                        flush(True)
                        # evict o_ps to SBUF promptly (ACT) so the PSUM bank
                        # frees without waiting on the rinv->normalize chain;
                        # normalize then runs 2x-mode from SBUF off-path
                        o_sb = fop.tile([P, 512], F16, tag="oraw")
                        nc.scalar.copy(o_sb[:], o_ps[:])
                        rinv = rvp.tile([P, 512], F32, tag="rinv")
                        nc.vector.reciprocal_approx_fast(out=rinv[:],
                                                         in_=r_ps[:])
                        if qc == 0:
                            nc.vector.tensor_mul(OT16[:, h, :], o_sb[:],
                                                 rinv[:])
                            # natural-scale fp8 copy of tokens 128-511 for
                            # the fp8 output projection of qi 1-3
                            nc.vector.tensor_mul(OT8[:, h, 128:512],
                                                 o_sb[:, 128:512],
                                                 rinv[:, 128:512])
                        else:
                            nc.vector.tensor_mul(OT8[:, h, ts(qc, 512)],
                                                 o_sb[:], rinv[:])
                        if h + 1 < HPC and qc == 0:
                            wq16n, wk16n, wq8n, wk8n = load_w(h + 1)
                            qraw_n = w1.tile([P, S], F16, tag="qraw")
                            kraw_n = w1.tile([P, S], F16, tag="kraw")
                            proj_corner(wq16n, qraw_n)
                            proj_corner(wk16n, kraw_n)
                            proj_fp8(wq8n, qraw_n, 3)
                            proj_fp8(wk8n, kraw_n, 3)
                            dchains.extend([(wq8n, qraw_n, 1),
                                            (wk8n, kraw_n, 1),
                                            (wq8n, qraw_n, 2),
                                            (wk8n, kraw_n, 2)])

                            def _rope(q=qraw_n, k=kraw_n, hn=h + 1):
                                rots[hn] = (rope(q, "qrot"),
                                            rope(k, "krot"))
                            ropef[0] = _rope
                    if h == HPC - 1:
                        for qi in range(12, 16):
                            emit_f8(qi, pjp, tail=True)

    nc.compile()
    return nc


def _prep_inputs_causal8(x, wq, wk, wv, wo, freqs_cos, freqs_sin):
    """Build the 8 per-core input maps for the fp8 causal kernel."""
    perm = np.concatenate([np.arange(0, HD, 2), np.arange(1, HD, 2)])

    cosT = freqs_cos.T.astype(np.float32)
    sinT = freqs_sin.T.astype(np.float32)
    c2 = np.concatenate([cosT, cosT], 0).astype(NPF16)
    s2n = np.concatenate([-sinT, sinT], 0).astype(NPF16)

    pp, ff = np.meshgrid(np.arange(P), np.arange(P), indexing="ij")
    mtri = np.where(pp > ff, MASK_NEG, 0.0).astype(NPF16)
    pp2, ff2 = np.meshgrid(np.arange(P), np.arange(256), indexing="ij")
    mtri2 = np.where(ff2 < pp2 + 128, MASK_NEG, 0.0).astype(NPF16)

    common = {
        "c2": c2, "s2n": s2n,
        "eye": np.eye(P, dtype=NPF16),
        "mtri": mtri, "mtri2": mtri2,
        "ones": np.ones((P, P), NPF16),
        "oneq": np.full((P, 2, P), 1.0 / OS, NPF8),
    }

    xT_by_b = []
    for b in range(B):
        xT = np.ascontiguousarray(
            x[b].T.reshape(KC, P, NQ, 512).transpose(1, 2, 0, 3))
        xT_by_b.append((np.ascontiguousarray(xT[:, 0]).astype(NPF16),
                        np.ascontiguousarray(xT[:, 1:]).astype(NPF8)))

    in_maps = []
    for c in range(NCORES):
        b, g = divmod(c, TP)
        heads = range(g * HPC, (g + 1) * HPC)
        cols_qk = np.concatenate([h * HD + perm for h in heads])
        cols_v = np.concatenate([np.arange(h * HD, (h + 1) * HD)
                                 for h in heads])

        wq_c = np.ascontiguousarray(
            wq[:, cols_qk].reshape(KC, P, HPC, HD).transpose(1, 2, 0, 3))
        wk_c = np.ascontiguousarray(
            wk[:, cols_qk].reshape(KC, P, HPC, HD).transpose(1, 2, 0, 3))
        wv_c = np.ascontiguousarray(
            wv[:, cols_v].reshape(KC, P, DVC).transpose(1, 0, 2))
        wo_c = np.ascontiguousarray(
            wo[cols_v, :].reshape(HPC, P, D).transpose(1, 0, 2))

        im = dict(common)
        im["xT16"], im["xT8"] = xT_by_b[b]
        im["wq16"] = wq_c.astype(NPF16)
        im["wk16"] = wk_c.astype(NPF16)
        im["wq8"] = (wq_c * WS).astype(NPF8)
        im["wk8"] = (wk_c * WS).astype(NPF8)
        im["wv16"] = wv_c.astype(NPF16)
        im["wv8"] = (wv_c * WS).astype(NPF8)
        im["wo16"] = wo_c.astype(NPF16)
        im["wo8"] = (wo_c * WS).astype(NPF8)
        in_maps.append(im)
    return in_maps


def _build(mask_mode: str):
    """Build + compile the SPMD program. mask_mode: 'causal'|'none'|'general'."""
    nc = bacc.Bacc("TRN2", target_bir_lowering=False, debug=False,
                   num_devices=NCORES)

    def din(name, shape, dt=F16):
        return nc.dram_tensor(name, shape, dt, kind="ExternalInput").ap()

    xT_d = din("xT", [P, NQ, KC, 512])
    wq_d = din("wq", [P, HPC, KC, HD])
    wk_d = din("wk", [P, HPC, KC, HD])
    wv_d = din("wv", [P, KC, DVC])
    wo_d = din("wo", [P, HPC, D])
    c2_d = din("c2", [P, S])
    s2n_d = din("s2n", [P, S])
    ones_d = din("ones", [P, P])
    if mask_mode == "causal":
        eye_d = din("eye", [P, P])
        mtri_d = din("mtri", [P, P])
    elif mask_mode == "general":
        msk_d = din("expm", [P, NT, S])
    out_d = nc.dram_tensor("out", [P, NT, D], mybir.dt.float16,
                           kind="ExternalOutput").ap()

    with tile.TileContext(nc) as tc:
        with tc.tile_pool(name="static", bufs=1) as st, \
             tc.tile_pool(name="w1", bufs=1) as w1, \
             tc.tile_pool(name="w2", bufs=2) as w2, \
             tc.tile_pool(name="et", bufs=6) as etp, \
             tc.tile_pool(name="ac", bufs=3) as accp, \
             tc.tile_pool(name="fo", bufs=4) as fop, \
             tc.tile_pool(name="pj", bufs=2, space="PSUM") as pjp:

            # ---- static tensors -------------------------------------------
            xT = st.tile([P, NQ, KC, 512], F16, tag="xT")
            wv_sb = st.tile([P, KC, DVC], F16, tag="wv")
            wo_sb = st.tile([P, HPC, D], F16, tag="wo")
            c2 = st.tile([P, S], F16, tag="c2")
            s2n = st.tile([P, S], F16, tag="s2n")
            ones_sb = st.tile([P, P], F16, tag="ones")
            V_sb = st.tile([P, NT, DVC], F16, tag="V")
            OT_sb = st.tile([P, HPC, S], F16, tag="OT")
            if mask_mode == "causal":
                eye_sb = st.tile([P, P], F16, tag="eye")
                mtri_sb = st.tile([P, P], F16, tag="mtri")
            bias4 = st.tile([P, 1], F32, tag="b4")
            nc.vector.memset(bias4[:], -4.0)

            # head-0 weights first (small), then interleaved wv/xT chunks so
            # the V-phase matmuls can start as soon as chunk 0 lands.
            wq_h = w1.tile([P, KC, HD], F16, tag="wqh")
            wk_h = w1.tile([P, KC, HD], F16, tag="wkh")
            for g in range(4):
                nc.sync.dma_start(wq_h[:, ts(g, 4), :], wq_d[:, 0, ts(g, 4), :])
                nc.sync.dma_start(xT[:, 0, ts(g, 4), :], xT_d[:, 0, ts(g, 4), :])
            nc.sync.dma_start(wk_h[:], wk_d[:, 0])
            nc.sync.dma_start(wv_sb[:], wv_d)
            nc.sync.dma_start(xT[:, 1, :, :], xT_d[:, 1, :, :])
            nc.sync.dma_start(xT[:, 2, :, :], xT_d[:, 2, :, :])
            nc.sync.dma_start(xT[:, 3, :, :], xT_d[:, 3, :, :])
            nc.sync.dma_start(c2[:], c2_d)
            nc.sync.dma_start(s2n[:], s2n_d)
            nc.sync.dma_start(ones_sb[:], ones_d)
            if mask_mode == "causal":
                nc.sync.dma_start(eye_sb[:], eye_d)
                nc.sync.dma_start(mtri_sb[:], mtri_d)
            nc.sync.dma_start(wo_sb[:], wo_d)

            def proj_half(w_h, raw_tag, rot_tag):
                """One projection (Q or K) + RoPE -> rotated [HD, S] tile."""
                raw = w1.tile([P, S], F16, tag=raw_tag)
                for t in range(NQ):
                    ps = pjp.tile([P, 512], F32, tag="pj")
                    for kc in range(KC):
                        nc.tensor.matmul(ps[:], w_h[:, kc, :],
                                         xT[:, t, kc, :],
                                         start=(kc == 0), stop=(kc == KC - 1))
                    nc.scalar.copy(raw[:, ts(t, 512)], ps[:])
                return rope(raw, rot_tag)

            def load_w(h):
                wq_h = w1.tile([P, KC, HD], F16, tag="wqh")
                nc.sync.dma_start(wq_h[:], wq_d[:, h])
                wk_h = w1.tile([P, KC, HD], F16, tag="wkh")
                nc.sync.dma_start(wk_h[:], wk_d[:, h])
                return wq_h, wk_h

            # head-0 projections first, Q/K interleaved per token chunk so the
            # PE consumes xT chunks as the startup DMAs land; the RoPE chains
            # then run during the V phase.
            def rope(raw, rot_tag):
                swp = w1.tile([P, S], F16, tag="swap")
                nc.vector.tensor_copy(swp[0:64, :], raw[64:128, :])
                nc.vector.tensor_copy(swp[64:128, :], raw[0:64, :])
                rot = w2.tile([P, S], F16, tag=rot_tag)
                nc.vector.tensor_mul(rot[:], raw[:], c2[:])
                nc.vector.tensor_mul(swp[:], swp[:], s2n[:])
                nc.vector.tensor_add(rot[:], rot[:], swp[:])
                return rot

            qraw0 = w1.tile([P, S], F16, tag="qraw")
            kraw0 = w1.tile([P, S], F16, tag="kraw")
            for t in range(NQ):
                for w_h, raw in ((wq_h, qraw0), (wk_h, kraw0)):
                    ps = pjp.tile([P, 512], F32, tag="pj")
                    for kc in range(KC):
                        nc.tensor.matmul(ps[:], w_h[:, kc, :], xT[:, t, kc, :],
                                         start=(kc == 0), stop=(kc == KC - 1))
                    nc.scalar.copy(raw[:, ts(t, 512)], ps[:])
                # V projection for this token chunk keeps the PE busy while
                # the next xT chunk is still streaming in
                for ti in range(4 * t, 4 * t + 4):
                    ps = pjp.tile([P, 512], F32, tag="pj")
                    for kc in range(KC):
                        nc.tensor.matmul(ps[:], xT[:, t, kc, ts(ti % 4, P)],
                                         wv_sb[:, kc, :],
                                         start=(kc == 0), stop=(kc == KC - 1))
                    nc.scalar.copy(V_sb[:, ti, :], ps[:])
            rots = {0: (rope(qraw0, "qrot"), rope(kraw0, "krot"))}

            flip = [False]

            def emit_f(qi, pool, scalar_only=False):
                for nn in range(D // 512):
                    ps = pool.tile([P, 512], F32, tag="pj")
                    for hh in range(HPC):
                        nc.tensor.matmul(ps[:], OT_sb[:, hh, ts(qi, P)],
                                         wo_sb[:, hh, ts(nn, 512)],
                                         start=(hh == 0), stop=(hh == HPC - 1))
                    f_sb = fop.tile([P, 512], F16, tag="fsb")
                    # keep the copies off the Vector engine when F is inlined
                    # into attention: they would head-of-line-block the acc
                    # adds in DVE's in-order queue
                    if flip[0] and not scalar_only:
                        nc.vector.tensor_copy(f_sb[:], ps[:])
                    else:
                        nc.scalar.copy(f_sb[:], ps[:])
                    flip[0] = not flip[0]
                    nc.sync.dma_start(out_d[:, qi, ts(nn, 512)], f_sb[:])

            # ---- attention, with next head's projections interleaved ------
            with tc.tile_pool(name="sp", bufs=3, space="PSUM") as stp, \
                 tc.tile_pool(name="op", bufs=2, space="PSUM") as opp, \
                 tc.tile_pool(name="rp", bufs=1, space="PSUM") as rpp:
                for h in range(HPC):
                    qrot, krot = rots.pop(h)
                    for qc in range(NQ):
                        o_ps = opp.tile([P, 512], F32, tag="o")
                        r_ps = rpp.tile([P, 512], F32, tag="r")
                        nkt = 4 * (qc + 1) if mask_mode == "causal" else NT
                        nfull = 4 * qc if mask_mode == "causal" else 0
                        # previous chunk's output-projection tiles, spread one
                        # per kt iteration so their PSUM->SBUF copies pace
                        # evenly through both engines' queues
                        fq = (list(range(4 * (qc - 1), 4 * qc))
                              if h == HPC - 1 and mask_mode == "causal" and qc > 0
                              else [])
                        # full-width tiles accumulate on the DVE into `acc`;
                        # one ones-matmul on the sum replaces one per tile.
                        acc = first_e = None
                        for kt in range(nkt):
                            band = mask_mode == "causal" and kt >= nfull
                            off = 128 * (kt - nfull) if band else 0
                            s_ps = stp.tile([P, 512], F32, tag="s")
                            nc.tensor.matmul(
                                s_ps[:, off:], krot[:, ts(kt, P)],
                                qrot[:, 512 * qc + off: 512 * (qc + 1)],
                                start=True, stop=not band)
                            if band:
                                nc.tensor.matmul(
                                    s_ps[:, off:off + 128], eye_sb[:], mtri_sb[:],
                                    start=False, stop=True)
                            eT = etp.tile([P, 512], F16, tag="e")
                            # bias -4 (softmax is shift-invariant; the ones-
                            # matmul denominator absorbs it) keeps exp outputs
                            # well inside fp16 range even for hot scores
                            nc.scalar.activation(eT[:, off:], s_ps[:, off:], EXP,
                                                 scale=SCALE, bias=bias4[:])
                            if mask_mode == "general":
                                em = etp.tile([P, 512], F16, tag="em")
                                nc.sync.dma_start(em[:], msk_d[:, kt, ts(qc, 512)])
                                nc.gpsimd.tensor_mul(eT[:], eT[:], em[:])
                            nc.tensor.matmul(o_ps[:, off:],
                                             V_sb[:, kt, ts(h, HD)], eT[:, off:],
                                             start=(kt == 0), stop=(kt == nkt - 1))
                            if mask_mode != "causal":
                                nc.tensor.matmul(r_ps[:], ones_sb[:], eT[:],
                                                 start=(kt == 0),
                                                 stop=(kt == nkt - 1))
                            elif not band:
                                if first_e is not None:
                                    acc = accp.tile([P, 512], F16, tag="acc")
                                    nc.vector.tensor_add(acc[:], first_e[:], eT[:])
                                    first_e = None
                                elif acc is not None:
                                    nc.vector.tensor_add(acc[:], acc[:], eT[:])
                                else:
                                    first_e = eT
                            elif acc is None and first_e is None:
                                acc = accp.tile([P, 512], F16, tag="acc")
                                nc.vector.tensor_copy(acc[:], eT[:])
                            elif first_e is not None:
                                acc = accp.tile([P, 512], F16, tag="acc")
                                nc.vector.tensor_add(acc[:], first_e[:], eT[:])
                                first_e = None
                            else:
                                nc.vector.tensor_add(acc[:, off:], acc[:, off:],
                                                     eT[:, off:])
                            if fq and kt % 3 == 2:
                                emit_f(fq.pop(0), pjp)
                        # leftover output-projection tiles go before the
                        # denominator matmul: they keep the PE busy while the
                        # Scalar engine finishes the trailing band exps
                        for qi in fq:
                            emit_f(qi, pjp)
                        if acc is not None:
                            nc.tensor.matmul(r_ps[:], ones_sb[:], acc[:],
                                             start=True, stop=True)
                        rinv = fop.tile([P, 512], F32, tag="rinv")
                        nc.vector.reciprocal_approx_fast(out=rinv[:], in_=r_ps[:])
                        nc.vector.tensor_mul(OT_sb[:, h, ts(qc, 512)], o_ps[:],
                                             rinv[:])
                        # pipeline the next head's projections + RoPE
                        if h + 1 < HPC and qc == 0:
                            wq_n, wk_n = load_w(h + 1)
                            rots[h + 1] = (proj_half(wq_n, "qraw", "qrot"),
                                           proj_half(wk_n, "kraw", "krot"))
                    if h == HPC - 1 and mask_mode == "causal":
                        for qi in range(4 * (NQ - 1), NT):
                            emit_f(qi, pjp)

            # ---- output projection for non-causal modes (causal streams it
            # inside the last head's attention) --------------------------------
            if mask_mode != "causal":
                with tc.tile_pool(name="fp", bufs=6, space="PSUM") as fpp:
                    for qi in range(NT):
                        emit_f(qi, fpp)

    nc.compile()
    return nc


def _get_program(mask_mode: str):
    if mask_mode not in _cache:
        if mask_mode == "causal":
            _cache[mask_mode] = _build_causal8()
        else:
            _cache[mask_mode] = _build(mask_mode)
    return _cache[mask_mode]


def _detect_mask_mode(mask: np.ndarray) -> str:
    m = mask.reshape(S, S)
    iu = np.triu_indices(S, 1)
    upper = m[iu]
    lower_ok = np.max(np.abs(np.tril(m))) == 0.0
    if lower_ok and upper.size and np.all(upper <= -1e8):
        return "causal"
    if np.max(np.abs(m)) == 0.0:
        return "none"
    return "general"


def _prep_inputs(x, wq, wk, wv, wo, freqs_cos, freqs_sin, mask, mask_mode):
    """Build the 8 per-core input maps (host-side sharding + layout)."""
    # within-head even/odd permutation so RoPE pairs land in partition halves
    perm = np.concatenate([np.arange(0, HD, 2), np.arange(1, HD, 2)])

    cosT = freqs_cos.T.astype(np.float32)          # [64, S]
    sinT = freqs_sin.T.astype(np.float32)
    c2 = np.concatenate([cosT, cosT], 0).astype(NPF16)     # [128, S]
    s2n = np.concatenate([-sinT, sinT], 0).astype(NPF16)
    ones = np.ones((P, P), NPF16)

    common = {"c2": c2, "s2n": s2n, "ones": ones}
    if mask_mode == "causal":
        common["eye"] = np.eye(P, dtype=NPF16)
        pp, ff = np.meshgrid(np.arange(P), np.arange(P), indexing="ij")
        common["mtri"] = np.where(pp > ff, MASK_NEG, 0.0).astype(NPF16)
    elif mask_mode == "general":
        m = mask.reshape(S, S).astype(np.float32)
        # eT[kt_tok, qt_tok] is multiplied by exp(SCALE * mask[qt_tok, kt_tok])
        expm = np.exp(SCALE * m.T).astype(NPF16)            # [k_tok, q_tok]
        common["expm"] = np.ascontiguousarray(
            expm.reshape(NT, P, S).transpose(1, 0, 2))

    xT_by_b = []
    for b in range(B):
        xT = np.ascontiguousarray(
            x[b].T.reshape(KC, P, NQ, 512).transpose(1, 2, 0, 3)).astype(NPF16)
        xT_by_b.append(xT)

    in_maps = []
    for c in range(NCORES):
        b, g = divmod(c, TP)
        heads = range(g * HPC, (g + 1) * HPC)
        cols_qk = np.concatenate([h * HD + perm for h in heads])
        cols_v = np.concatenate([np.arange(h * HD, (h + 1) * HD) for h in heads])

        wq_c = wq[:, cols_qk].reshape(KC, P, HPC, HD).transpose(1, 2, 0, 3)
        wk_c = wk[:, cols_qk].reshape(KC, P, HPC, HD).transpose(1, 2, 0, 3)
        wv_c = wv[:, cols_v].reshape(KC, P, DVC).transpose(1, 0, 2)
        wo_c = wo[cols_v, :].reshape(HPC, P, D).transpose(1, 0, 2)

        im = dict(common)
        im["xT"] = xT_by_b[b]
        im["wq"] = np.ascontiguousarray(wq_c).astype(NPF16)
        im["wk"] = np.ascontiguousarray(wk_c).astype(NPF16)
        im["wv"] = np.ascontiguousarray(wv_c).astype(NPF16)
        im["wo"] = np.ascontiguousarray(wo_c).astype(NPF16)
        in_maps.append(im)
    return in_maps


def run(inputs: dict, **spmd_kwargs):
    """Run on hardware; returns (output [B,S,D] fp32, BassKernelResults)."""
    x = np.asarray(inputs["x"], np.float32)
    wq = np.asarray(inputs["wq"], np.float32)
    wk = np.asarray(inputs["wk"], np.float32)
    wv = np.asarray(inputs["wv"], np.float32)
    wo = np.asarray(inputs["wo"], np.float32)
    fc = np.asarray(inputs["freqs_cos"], np.float32)
    fs = np.asarray(inputs["freqs_sin"], np.float32)
    mask = np.asarray(inputs["mask"], np.float32)

    mask_mode = _detect_mask_mode(mask)
    nc = _get_program(mask_mode)
    if mask_mode == "causal":
        in_maps = _prep_inputs_causal8(x, wq, wk, wv, wo, fc, fs)
    else:
        in_maps = _prep_inputs(x, wq, wk, wv, wo, fc, fs, mask, mask_mode)
    res = run_bass_kernel_spmd(nc, in_maps, core_ids=list(range(NCORES)),
                               **spmd_kwargs)

    out = np.zeros((B, S, D), np.float32)
    for c in range(NCORES):
        b = c // TP
        part = res.results[c]["out"].astype(np.float32)   # [P, NT, D]
        out[b] += part.transpose(1, 0, 2).reshape(S, D)
    return out, res


def kernel(**inputs) -> np.ndarray:
    out, _ = run(inputs)
    return out

